# revision 35
# baseline (speedup 1.0000x reference)
"""Trainium2 Bass kernel for GRNNTransformSimple (bottom-up binary-tree GRNN).

Computation (per jet): heap-layout complete binary tree, DEPTH=14.
  u_k   = relu(contents_k @ Wu + bu)                         (all nodes)
  emb_k = u_k                                                (leaves)
  emb_k = relu(hL @ Wh[:64] + hR @ Wh[64:128] + u_k @ Wh[128:] + bh)  (inner)
Output: root emb, [B, 64].

Mapping (8 NeuronCores, data-parallel over B=128 jets, 16 jets/core):
 - 2 jets packed per 128 SBUF partitions (jet A on partitions 0-63, jet B on
   64-127) with block-diagonal weights -> all engines run 128 partitions wide.
 - The PE streams exactly 1 output column per cycle in every mode
   (measured: bf16 = fp8 = fp8-DoubleRow = 216ns per 512-col matmul), so
   the only matmul-count lever is fusing the L and R contributions into a
   single pass: fp8e4m3 DoubleRow with stationary planes (WhL_bd, WhR_bd)
   and moving planes (hL, hR) = the even/odd column interleave of the
   child level. Applied at the deep levels (12..10, 87% of fc_h work).
   Deep-level fp8 quantization noise averages out on the way up the tree
   (measured: rel_rms 4.1e-3 vs 4.1e-3 pure-bf16 on the reference inputs);
   fp8 on the top levels does NOT average and is left in bf16.
 - fc_u biases folded into the matmul via a constant-one input row (K=18),
   4-way strip-tiled (tile_position) since K=18 << 128. Each 2048-col
   stream window is issued as a WAVE of 4 consecutive strip matmuls at
   different tile positions: the PE's 16x 32x32 sub-arrays execute them
   CONCURRENTLY (measured 53ns/matmul vs 216 serial, ~4x), so fc_u drops
   from 51us to ~13us of PE time. The deep fc_u stream (c4 + its weights)
   is fp8 (halves the startup DMA and SBUF reads; same PE speed).
 - fc_h U-term stays bf16 (same pass count as fp8, cheap fast-weight-load).
 - Levels 9..0 are batched across pairs, split into two independent 4-pair
   chains so one chain's matmuls fill the other's activation stalls;
   levels 9..8 use the fused fp8 L+R, 7..0 bf16 (top-level u stays bf16).
 - The "irregular" child gather is regular for arange children: children of
   level-i node j are nodes 2j, 2j+1 of level i+1, i.e. stride-2 column
   slices (the fp8 DoubleRow plane view) of the level-(i+1) embedding.
 - Pair 0's c4 is DMA'd in 128KB column chunks so the first wave starts
   ~1us in; later pairs use one DMA each (Sync-engine issue cost, ~600ns
   per DMA, dominates over transfer bandwidth). The u phase for levels
   0..9 (utop) is emitted last to fill tail stalls.
 - relu activations are split between the Scalar and Vector engines by a
   greedy balance on estimated cost; with the PE tricks above the act
   engines are co-critical with the PE (~110us busy each).
"""

import os
import sys

sys.path.insert(0, "/opt/trn_rl_repo")

import ml_dtypes
import numpy as np

DEPTH = 14
B = 128
F = 8
H = 64
N_NODES = 2**DEPTH - 1  # 16383
N_INNER = 2 ** (DEPTH - 1) - 1  # 8191
N_CORES = 8
JPC = 16  # jets per core
NPAIR = 8  # jet pairs per core

BF16 = ml_dtypes.bfloat16
FP8 = ml_dtypes.float8_e4m3fn

# u_stream layout per pair (columns): levels 10,11,12 inner nodes in heap
# order, then all leaves in heap order.
UB10, UB11, UB12 = 0, 1024, 3072  # level bases inside u_stream
ULEAF = 7168
USTREAM = 15360  # 1024 + 2048 + 4096 + 8192
NGRP = 15  # 15 groups x 1024 cols
# u_top: levels 0..9, column order [level][pair][node]
UTOP_COLS = 8184  # 8 * 1023
UTOP_PAD = 8192


def _np_reference(contents, children, Wu, bu, Wh, bh):
    emb = None
    for i in range(DEPTH - 1, -1, -1):
        off, n = 2**i - 1, 2**i
        u = np.maximum(contents[:, off : off + n] @ Wu + bu, 0)
        if emb is None:
            emb = u
        else:
            ch = children[off : off + n] - 2 * off
            hL = emb[:, ch[:, 0]]
            hR = emb[:, ch[:, 1]]
            emb = np.maximum(
                hL @ Wh[:H] + hR @ Wh[H : 2 * H] + u @ Wh[2 * H :] + bh, 0
            )
    return emb.reshape(emb.shape[0], -1).astype(np.float32)


def _strip_pack(S):
    """Pack a [18, N] stream into the [128, ceil(N/4096)*1024] strip layout:
    wave (j, h) covers the contiguous 2048-col stream window starting at
    4096j + 2048h; its 4 strips (rows 32t..32t+17, tile cols
    [1024j + 512h : +512]) hold the window's four 512-col quarters. The 4
    strip matmuls of a wave then run concurrently in the PE sub-arrays and
    fill one [128, 2048] PSUM tile that drains with a single activation."""
    n = S.shape[1]
    out = np.zeros((128, ((n + 4095) // 4096) * 1024), dtype=S.dtype)
    for w0 in range(0, n, 2048):  # wave window
        j, h = w0 // 4096, (w0 % 4096) // 2048
        for t in range(4):
            s0 = w0 + 512 * t
            if s0 >= n:
                break
            s1 = min(s0 + 512, n)
            c0 = 1024 * j + 512 * h
            out[32 * t : 32 * t + 18, c0 : c0 + (s1 - s0)] = S[:, s0:s1]
    return out


def _prep_core_inputs(contents):
    """contents: [16, 16383, 8] f32 for one core.
    Returns dict of per-core device input arrays."""
    c4 = np.zeros((NPAIR, 128, 4096), dtype=FP8)
    big_T = np.ascontiguousarray(
        np.transpose(contents[:, 1023:16383, :], (0, 2, 1))
    )  # [16, 8, 15360]
    for p in range(NPAIR):
        S = np.empty((18, USTREAM), dtype=np.float32)
        S[0:8] = big_T[2 * p]
        S[8] = 1.0
        S[9:17] = big_T[2 * p + 1]
        S[17] = 1.0
        c4[p] = _strip_pack(S.astype(FP8))

    # u_top stream: [level][pair][node]
    tops = np.empty((18, UTOP_COLS), dtype=np.float32)
    colptr = 0
    cT = np.transpose(contents, (0, 2, 1))  # [16, 8, 16383]
    for i in range(10):
        off, n = 2**i - 1, 2**i
        for p in range(NPAIR):
            tops[0:8, colptr : colptr + n] = cT[2 * p][:, off : off + n]
            tops[8, colptr : colptr + n] = 1.0
            tops[9:17, colptr : colptr + n] = cT[2 * p + 1][:, off : off + n]
            tops[17, colptr : colptr + n] = 1.0
            colptr += n
    assert colptr == UTOP_COLS
    tb = np.zeros((18, UTOP_PAD), dtype=BF16)
    tb[:, :UTOP_COLS] = tops.astype(BF16)
    ctop = _strip_pack(tb)
    return {"c4": c4, "ctop": ctop}


def _prep_weights(Wu, bu, Wh, bh):
    wu2 = np.zeros((18, 128), dtype=np.float32)
    wu2[0:8, 0:64] = Wu
    wu2[8, 0:64] = bu
    wu2[9:17, 64:128] = Wu
    wu2[17, 64:128] = bu
    # fc_u stationary, 4-way strip layout: fp8 for the deep stream,
    # bf16 for the top-levels (ctop) stream
    wub = np.zeros((128, 128), dtype=BF16)
    wu8 = np.zeros((128, 128), dtype=FP8)
    for t in range(4):
        wub[32 * t : 32 * t + 18, :] = wu2.astype(BF16)
        wu8[32 * t : 32 * t + 18, :] = wu2.astype(FP8)

    def blockdiag(Wx):
        out = np.zeros((128, 128), dtype=np.float32)
        out[0:64, 0:64] = Wx
        out[64:128, 64:128] = Wx
        return out

    bdl = blockdiag(Wh[0:H])
    bdr = blockdiag(Wh[H : 2 * H])
    bdu = blockdiag(Wh[2 * H : 3 * H])
    whlr8 = np.concatenate([bdl, bdr], axis=1).astype(FP8)
    bh2 = np.concatenate([bh, bh]).astype(np.float32).reshape(128, 1)
    return {
        "wub": wub,
        "wu8": wu8,
        "whlr8": whlr8,
        "whlb": bdl.astype(BF16),
        "whrb": bdr.astype(BF16),
        "whub": bdu.astype(BF16),
        "bh2": bh2,
    }


def _dedup_ldweights(nc):
    """Delete an LDWEIGHTS whose signature matches the last weight load FOR
    ITS TILE POSITION (only MATMULs in between): the 128x128 PE array is 16
    independent 32x32 sub-arrays, so each tile position keeps its stationary
    operand resident until an overlapping load clobbers it. Sync info of
    deleted loads is merged into the following PE instruction."""
    n_del = 0

    def row_range(inst):
        pos = inst.tile_position
        size = inst.tile_size
        r0 = pos[0] if pos else 0
        nr = size[0] if size else 128
        return (r0, r0 + nr)

    for f in nc.m.functions:
        for bb in f.blocks:
            last_by_pos = {}  # (r0, r1) -> sig
            pending_sync = None
            out = []
            for inst in bb.instructions:
                tn = type(inst).__name__
                if str(getattr(inst, "engine", "")) == "EngineType.PE":
                    if tn == "InstLdweights":
                        a = inst.ins[0]
                        rr = row_range(inst)
                        sig = (
                            getattr(a, "memref", None),
                            getattr(a, "offset", None),
                            str(getattr(a, "ap", None)),
                            str(inst.tile_position),
                            str(inst.tile_size),
                            str(inst.perf_mode),
                            str(inst.is_transpose),
                        )
                        if last_by_pos.get(rr) == sig:
                            n_del += 1
                            si = inst.sync_info
                            if si is not None and (si.on_wait or si.on_update):
                                if pending_sync is None:
                                    pending_sync = ([], [])
                                pending_sync[0].extend(si.on_wait)
                                pending_sync[1].extend(si.on_update)
                            continue  # drop this instruction
                        # clobber any overlapping row range, then record
                        for orr in [
                            k
                            for k in last_by_pos
                            if k[0] < rr[1] and rr[0] < k[1] and k != rr
                        ]:
                            del last_by_pos[orr]
                        last_by_pos[rr] = sig
                    elif tn != "InstMatmult":
                        last_by_pos.clear()  # anything else on PE invalidates
                    if pending_sync is not None:
                        si = inst.sync_info
                        if si is None:
                            import concourse.mybir as mybir

                            inst.sync_info = mybir.SyncInfo(
                                on_wait=list(pending_sync[0]),
                                on_update=list(pending_sync[1]),
                            )
                        else:
                            si.on_wait[:0] = pending_sync[0]
                            si.on_update.extend(pending_sync[1])
                        pending_sync = None
                out.append(inst)
            assert pending_sync is None, "dangling sync from deleted trailing LDW"
            bb.instructions.clear()
            for i in out:
                bb.add_instruction(i)
    return n_del


def _split_sync_waits(nc, mybir, max_waits=1):
    """This container's walrus only accepts 1 sync-wait per instruction;
    move excess waits onto preceding same-engine NoOps."""
    for f in nc.m.functions:
        for bb in f.blocks:
            out = []
            for inst in bb.instructions:
                si = inst.sync_info
                if si is not None and len(si.on_wait) > max_waits:
                    waits = list(si.on_wait)
                    extra, keep = waits[:-max_waits], waits[-max_waits:]
                    for i in range(0, len(extra), max_waits):
                        nop = mybir.InstNoOp(
                            name=nc.get_next_instruction_name(),
                            engine=inst.engine,
                            sync_info=mybir.SyncInfo(
                                on_wait=extra[i : i + max_waits], on_update=[]
                            ),
                        )
                        out.append(nop)
                    si.on_wait = keep
                out.append(inst)
            bb.instructions.clear()
            for i in out:
                bb.add_instruction(i)


def _build_nc():
    import concourse.bass as bass
    import concourse.mybir as mybir
    from concourse.tile import TileContext

    fp32 = mybir.dt.float32
    bf16 = mybir.dt.bfloat16
    fp8 = mybir.dt.float8e4
    RELU = mybir.ActivationFunctionType.Relu
    ADD = mybir.AluOpType.add
    MAX = mybir.AluOpType.max
    DR = mybir.MatmulPerfMode.DoubleRow

    nc = bass.Bass(trn_type="TRN2", num_devices=N_CORES)
    c4_d = nc.dram_tensor("c4", [NPAIR, 128, 4096], fp8, kind="ExternalInput")
    ctop_d = nc.dram_tensor("ctop", [128, 2048], bf16, kind="ExternalInput")
    wub_d = nc.dram_tensor("wub", [128, 128], bf16, kind="ExternalInput")
    wu8_d = nc.dram_tensor("wu8", [128, 128], fp8, kind="ExternalInput")
    whlr8_d = nc.dram_tensor("whlr8", [128, 256], fp8, kind="ExternalInput")
    whlb_d = nc.dram_tensor("whlb", [128, 128], bf16, kind="ExternalInput")
    whrb_d = nc.dram_tensor("whrb", [128, 128], bf16, kind="ExternalInput")
    whub_d = nc.dram_tensor("whub", [128, 128], bf16, kind="ExternalInput")
    bh2_d = nc.dram_tensor("bh2", [128, 1], fp32, kind="ExternalInput")
    out_d = nc.dram_tensor("out", [128, NPAIR], fp32, kind="ExternalOutput")

    # greedy act-engine balance: [scalar(ACT), vector(DVE)] cumulative ns
    eng_load = [0.0, 0.0]

    with TileContext(nc) as tc:
        with (
            tc.tile_pool(name="wpool", bufs=1) as wpool,
            tc.tile_pool(name="c4pool", bufs=3) as c4pool,
            tc.tile_pool(name="uspool", bufs=4) as uspool,
            tc.tile_pool(name="e12pool", bufs=2) as e12pool,
            tc.tile_pool(name="e11pool", bufs=2) as e11pool,
            tc.tile_pool(name="shpool", bufs=1) as shpool,
            tc.tile_pool(name="pspool", bufs=4, space="PSUM") as pspool,
        ):
            whlr_sb = wpool.tile([128, 256], fp8, tag="whlr")
            bh_sb = wpool.tile([128, 1], fp32, tag="bh")
            wub_sb = wpool.tile([128, 128], bf16, tag="wub")
            wu8_sb = wpool.tile([128, 128], fp8, tag="wu8")
            whlb_sb = wpool.tile([128, 128], bf16, tag="whlb")
            whrb_sb = wpool.tile([128, 128], bf16, tag="whrb")
            whub_sb = wpool.tile([128, 128], bf16, tag="whub")
            ctop_sb = wpool.tile([128, 2048], bf16, tag="ctop")
            utop = wpool.tile([128, UTOP_PAD], bf16, tag="utop")
            # critical-path weights first
            nc.sync.dma_start(wu8_sb[:], wu8_d.ap())
            nc.sync.dma_start(wub_sb[:], wub_d.ap())
            nc.sync.dma_start(whlr_sb[:], whlr8_d.ap())
            nc.sync.dma_start(whub_sb[:], whub_d.ap())
            nc.sync.dma_start(bh_sb[:], bh2_d.ap())

            whlr_v = whlr_sb[:, 0:256].rearrange("p (two m) -> p two m", two=2)

            def act_relu(dst_ap, src_ap, bias, ncols):
                """relu(src + bias) -> dst on the act engine with the least
                estimated accumulated load."""
                cost = (230.0 + 0.833 * ncols, 147.0 + 1.042 * ncols)
                e = 0 if eng_load[0] + cost[0] <= eng_load[1] + cost[1] else 1
                eng_load[e] += cost[e]
                if e == 0:
                    if bias is None:
                        nc.scalar.activation(dst_ap, src_ap, RELU)
                    else:
                        nc.scalar.activation(dst_ap, src_ap, RELU, bias=bias)
                else:
                    if bias is None:
                        nc.vector.tensor_scalar(dst_ap, src_ap, 0.0, None, MAX)
                    else:
                        nc.vector.tensor_scalar(dst_ap, src_ap, bias, 0.0, ADD, MAX)

            def u_phase(p, chunks, ustrb, ustr8):
                """fc_u for the deep stream of one pair (bf16, strip-tiled).
                Each wave = 4 matmuls at different tile_positions, run
                CONCURRENTLY by the PE's 32-row sub-arrays (~4x throughput),
                filling the four 512-col quarters of ONE [128, 2048] PSUM
                tile = one contiguous stream window = one activation.
                Inner-node u (cols < 7168) lands in bf16 for the U-term;
                leaf u lands in fp8 for the level-12 fused L+R."""
                for w in range(8):
                    s0 = 2048 * w
                    nstrip = min(4, (USTREAM - s0 + 511) // 512)
                    ch, cb = chunks[w // 2]
                    hc = cb + 512 * (w % 2)
                    # wave of up-to-4 concurrent strip matmuls filling two
                    # [128, 1024] psum tiles (strips 0-1 / 2-3)
                    pss = [
                        pspool.tile(
                            [128, 1024], fp32, tag="ps", name=f"psu{p}_{w}_{k}"
                        )
                        for k in range((nstrip + 1) // 2)
                    ]
                    for t in range(nstrip):
                        nc.tensor.matmul(
                            pss[t // 2][:, 512 * (t % 2) : 512 * (t % 2 + 1)],
                            wu8_sb[32 * t : 32 * t + 18, :],
                            ch[32 * t : 32 * t + 18, hc : hc + 512],
                            start=True,
                            stop=True,
                            tile_position=(32 * t, 0),
                        )
                    for k in range((nstrip + 1) // 2):
                        a0 = s0 + 1024 * k
                        a1 = min(a0 + 1024, s0 + 512 * nstrip)
                        if a0 < ULEAF:
                            dstt, base = ustrb, 0
                        else:
                            dstt, base = ustr8, ULEAF
                        act_relu(
                            dstt[:, a0 - base : a1 - base],
                            pss[k][:, 0 : a1 - a0],
                            None,
                            a1 - a0,
                        )

            def levels_deep(p, ustrb, ustr8, emb10sh):
                """fc_h levels 12..10 for one pair: fused L+R via fp8
                DoubleRow, U-term in bf16, 2048-col supergroups -> one act
                per supergroup and 2 weight switches."""
                emb12 = e12pool.tile([128, 4096], fp8, tag="e12")
                emb11 = e11pool.tile([128, 2048], fp8, tag="e11")
                for i, ubase, prev, dst, dst_base in (
                    (12, UB12, ustr8, emb12, 0),
                    (11, UB11, emb12, emb11, 0),
                    (10, UB10, emb11, emb10sh, 1024 * p),
                ):
                    m = 2**i
                    for s0 in range(0, m, 1024):
                        w = min(1024, m - s0)
                        ps = pspool.tile(
                            [128, 1024], fp32, tag="ps", name=f"psl{p}_{i}_{s0}"
                        )
                        for h0 in range(0, w, 512):
                            j0 = s0 + h0
                            mv = prev[:, 2 * j0 : 2 * j0 + 1024].rearrange(
                                "p (n two) -> p two n", two=2
                            )
                            nc.tensor.matmul(
                                ps[:, h0 : h0 + 512],
                                whlr_v,
                                mv,
                                start=True,
                                stop=False,
                                perf_mode=DR,
                            )
                        for h0 in range(0, w, 512):
                            j0 = s0 + h0
                            nc.tensor.matmul(
                                ps[:, h0 : h0 + 512],
                                whub_sb[:],
                                ustrb[:, ubase + j0 : ubase + j0 + 512],
                                start=False,
                                stop=True,
                            )
                        act_relu(
                            dst[:, dst_base + s0 : dst_base + s0 + w],
                            ps[:, 0:w],
                            bh_sb[:],
                            w,
                        )

            # ---- pairs, software-pipelined: u(p) emitted before levels(p-1)
            # so independent u work fills the level chains' act stalls ----
            emb10sh = shpool.tile([128, 8192], fp8, tag="e10")
            ustrbs = [None] * NPAIR
            ustr8s = [None] * NPAIR
            for p in range(NPAIR):
                chunks = []
                if p == 0:
                    # chunked first pair so the first wave starts ~1us in
                    for j in range(4):
                        ch = c4pool.tile(
                            [128, 1024], fp8, tag="c4", name=f"c4_{p}_{j}"
                        )
                        nc.sync.dma_start(
                            ch[:], c4_d.ap()[p][:, 1024 * j : 1024 * (j + 1)]
                        )
                        chunks.append((ch, 0))
                else:
                    # one DMA per pair: issue cost on the Sync engine is the
                    # startup bottleneck, not transfer bandwidth
                    whole = c4pool.tile([128, 4096], fp8, tag="c4w", name=f"c4w{p}")
                    nc.sync.dma_start(whole[:], c4_d.ap()[p])
                    chunks = [(whole, 1024 * j) for j in range(4)]
                ustrbs[p] = uspool.tile([128, 7168], bf16, tag="usb", name=f"ustrb{p}")
                ustr8s[p] = uspool.tile([128, 8192], fp8, tag="us8", name=f"ustr8{p}")
                u_phase(p, chunks, ustrbs[p], ustr8s[p])
                if p == 0:
                    # tail-phase inputs; after the critical first chunk
                    nc.sync.dma_start(whlb_sb[:], whlb_d.ap())
                    nc.sync.dma_start(whrb_sb[:], whrb_d.ap())
                    nc.sync.dma_start(ctop_sb[:], ctop_d.ap())
                if p > 0:
                    levels_deep(p - 1, ustrbs[p - 1], ustr8s[p - 1], emb10sh)
            levels_deep(NPAIR - 1, ustrbs[NPAIR - 1], ustr8s[NPAIR - 1], emb10sh)

            # ---- u for levels 0..9 (strip waves), emitted late as filler ----
            for w in range(4):
                cc = 1024 * (w // 2) + 512 * (w % 2)
                pss = [
                    pspool.tile([128, 1024], fp32, tag="ps", name=f"psut{w}_{k}")
                    for k in range(2)
                ]
                for t in range(4):
                    nc.tensor.matmul(
                        pss[t // 2][:, 512 * (t % 2) : 512 * (t % 2 + 1)],
                        wub_sb[32 * t : 32 * t + 18, :],
                        ctop_sb[32 * t : 32 * t + 18, cc : cc + 512],
                        start=True,
                        stop=True,
                        tile_position=(32 * t, 0),
                    )
                for k in range(2):
                    act_relu(
                        utop[:, 2048 * w + 1024 * k : 2048 * w + 1024 * (k + 1)],
                        pss[k][:, 0:1024],
                        None,
                        1024,
                    )

            # ---- levels 9..1 in two independent 4-pair chains ----
            # Levels 9..8 use the fused fp8 L+R (inputs emb10sh/esh9 are fp8);
            # levels 7..1 are bf16 (fp8 noise near the root does not average).
            eshs = [{}, {}]
            for i in range(9, 0, -1):
                m4 = 4 * (2**i)
                b8 = 8 * (2**i - 1)
                fused = i >= 8
                for X in range(2):
                    prev = emb10sh if i == 9 else eshs[X][i + 1]
                    pb = 4096 * X if i == 9 else 0
                    cur = wpool.tile(
                        [128, m4], fp8 if i == 9 else bf16, tag=f"esh{X}_{i}"
                    )
                    eshs[X][i] = cur
                    for s0 in range(0, m4, 1024):
                        w = min(1024, m4 - s0)
                        ps = pspool.tile(
                            [128, 1024], fp32, tag="ps", name=f"pst{X}_{i}_{s0}"
                        )
                        if fused:
                            for h0 in range(0, w, 512):
                                n = min(512, w - h0)
                                j0 = s0 + h0
                                mv = prev[
                                    :, pb + 2 * j0 : pb + 2 * j0 + 2 * n
                                ].rearrange("p (n two) -> p two n", two=2)
                                nc.tensor.matmul(
                                    ps[:, h0 : h0 + n],
                                    whlr_v,
                                    mv,
                                    start=True,
                                    stop=False,
                                    perf_mode=DR,
                                )
                            for h0 in range(0, w, 512):
                                n = min(512, w - h0)
                                j0 = s0 + h0
                                nc.tensor.matmul(
                                    ps[:, h0 : h0 + n],
                                    whub_sb[:],
                                    utop[:, b8 + m4 * X + j0 : b8 + m4 * X + j0 + n],
                                    start=False,
                                    stop=True,
                                )
                        else:
                            for w_sb, kind in (
                                (whlb_sb, "L"),
                                (whrb_sb, "R"),
                                (whub_sb, "U"),
                            ):
                                for h0 in range(0, w, 512):
                                    n = min(512, w - h0)
                                    j0 = s0 + h0
                                    if kind == "L":
                                        mv = prev[
                                            :, pb + 2 * j0 : pb + 2 * j0 + 2 * n : 2
                                        ]
                                    elif kind == "R":
                                        mv = prev[
                                            :,
                                            pb + 2 * j0 + 1 : pb + 2 * j0 + 2 * n : 2,
                                        ]
                                    else:
                                        mv = utop[
                                            :, b8 + m4 * X + j0 : b8 + m4 * X + j0 + n
                                        ]
                                    nc.tensor.matmul(
                                        ps[:, h0 : h0 + n],
                                        w_sb[:],
                                        mv,
                                        start=(kind == "L"),
                                        stop=(kind == "U"),
                                    )
                        act_relu(cur[:, s0 : s0 + w], ps[:, 0:w], bh_sb[:], w)

            # ---- level 0: roots, one per chain ----
            roots = wpool.tile([128, NPAIR], fp32, tag="roots")
            for X in range(2):
                ps = pspool.tile([128, 1024], fp32, tag="ps", name=f"psroot{X}")
                o = ps[:, 0:4]
                e1 = eshs[X][1]
                nc.tensor.matmul(o, whlb_sb[:], e1[:, 0:8:2], start=True, stop=False)
                nc.tensor.matmul(o, whrb_sb[:], e1[:, 1:8:2], start=False, stop=False)
                nc.tensor.matmul(
                    o, whub_sb[:], utop[:, 4 * X : 4 * X + 4], start=False, stop=True
                )
                nc.scalar.activation(
                    roots[:, 4 * X : 4 * X + 4], o, RELU, bias=bh_sb[:]
                )
            nc.sync.dma_start(out_d.ap(), roots[:])

    _dedup_ldweights(nc)
    _split_sync_waits(nc, mybir)
    return nc


_NC_CACHE = None
LAST_RESULTS = None


def kernel(contents, children, Wu, bu, Wh, bh):
    global _NC_CACHE, LAST_RESULTS
    contents = np.asarray(contents, dtype=np.float32)
    children = np.asarray(children)
    Wu = np.asarray(Wu, dtype=np.float32)
    bu = np.asarray(bu, dtype=np.float32)
    Wh = np.asarray(Wh, dtype=np.float32)
    bh = np.asarray(bh, dtype=np.float32)

    regular = (
        contents.shape == (B, N_NODES, F)
        and children.shape == (N_INNER, 2)
        and np.array_equal(
            np.asarray(children, dtype=np.int64).ravel(), np.arange(N_INNER * 2)
        )
    )
    if not regular:
        # Safety net for non-arange children: exact numpy fallback.
        return _np_reference(contents, children, Wu, bu, Wh, bh)

    from concourse.bass_utils import run_bass_kernel_spmd

    if _NC_CACHE is None:
        _NC_CACHE = _build_nc()
    nc = _NC_CACHE

    wts = _prep_weights(Wu, bu, Wh, bh)
    in_maps = []
    for k in range(N_CORES):
        m = _prep_core_inputs(contents[JPC * k : JPC * (k + 1)])
        m.update(wts)
        in_maps.append(m)

    res = run_bass_kernel_spmd(
        nc,
        in_maps,
        core_ids=list(range(N_CORES)),
        trace=bool(os.environ.get("BASS_TRACE")),
    )
    LAST_RESULTS = res

    out = np.empty((B, H), dtype=np.float32)
    for k in range(N_CORES):
        r = res.results[k]["out"].reshape(2, 64, NPAIR)  # [half, h, pair]
        out[JPC * k : JPC * (k + 1)] = np.transpose(r, (2, 0, 1)).reshape(JPC, H)
    return out


# revision 38
# speedup vs baseline: 1.0169x; 1.0169x over previous
"""Trainium2 Bass kernel for GRNNTransformSimple (bottom-up binary-tree GRNN).

Computation (per jet): heap-layout complete binary tree, DEPTH=14.
  u_k   = relu(contents_k @ Wu + bu)                         (all nodes)
  emb_k = u_k                                                (leaves)
  emb_k = relu(hL @ Wh[:64] + hR @ Wh[64:128] + u_k @ Wh[128:] + bh)  (inner)
Output: root emb, [B, 64].

Mapping (8 NeuronCores, data-parallel over B=128 jets, 16 jets/core):
 - 2 jets packed per 128 SBUF partitions (jet A on partitions 0-63, jet B on
   64-127) with block-diagonal weights -> all engines run 128 partitions wide.
 - The PE streams exactly 1 output column per cycle in every mode
   (measured: bf16 = fp8 = fp8-DoubleRow = 216ns per 512-col matmul), so
   the only matmul-count lever is fusing the L and R contributions into a
   single pass: fp8e4m3 DoubleRow with stationary planes (WhL_bd, WhR_bd)
   and moving planes (hL, hR) = the even/odd column interleave of the
   child level. Applied at the deep levels (12..10, 87% of fc_h work).
   Deep-level fp8 quantization noise averages out on the way up the tree
   (measured: rel_rms 4.1e-3 vs 4.1e-3 pure-bf16 on the reference inputs);
   fp8 on the top levels does NOT average and is left in bf16.
 - fc_u biases folded into the matmul via a constant-one input row (K=18),
   4-way strip-tiled (tile_position) since K=18 << 128. Each 2048-col
   stream window is issued as a WAVE of 4 consecutive strip matmuls at
   different tile positions: the PE's 16x 32x32 sub-arrays execute them
   CONCURRENTLY (measured 53ns/matmul vs 216 serial, ~4x), so fc_u drops
   from 51us to ~13us of PE time. The deep fc_u stream (c4 + its weights)
   is fp8 (halves the startup DMA and SBUF reads; same PE speed).
 - fc_h U-term stays bf16 (same pass count as fp8, cheap fast-weight-load).
 - Levels 9..0 are batched across pairs, split into two independent 4-pair
   chains so one chain's matmuls fill the other's activation stalls;
   levels 9..8 use the fused fp8 L+R, 7..0 bf16 (top-level u stays bf16).
 - The "irregular" child gather is regular for arange children: children of
   level-i node j are nodes 2j, 2j+1 of level i+1, i.e. stride-2 column
   slices (the fp8 DoubleRow plane view) of the level-(i+1) embedding.
 - Pair 0's c4 is DMA'd in 128KB column chunks so the first wave starts
   ~1us in; later pairs use one DMA each (Sync-engine issue cost, ~600ns
   per DMA, dominates over transfer bandwidth). The u phase for levels
   0..9 (utop) is emitted last to fill tail stalls.
 - relu activations are split between the Scalar and Vector engines by a
   greedy balance on estimated cost; with the PE tricks above the act
   engines are co-critical with the PE (~110us busy each).
"""

import os
import sys

sys.path.insert(0, "/opt/trn_rl_repo")

import ml_dtypes
import numpy as np

DEPTH = 14
B = 128
F = 8
H = 64
N_NODES = 2**DEPTH - 1  # 16383
N_INNER = 2 ** (DEPTH - 1) - 1  # 8191
N_CORES = 8
JPC = 16  # jets per core
NPAIR = 8  # jet pairs per core

BF16 = ml_dtypes.bfloat16
FP8 = ml_dtypes.float8_e4m3fn

# u_stream layout per pair (columns): levels 10,11,12 inner nodes in heap
# order, then all leaves in heap order.
UB10, UB11, UB12 = 0, 1024, 3072  # level bases inside u_stream
ULEAF = 7168
USTREAM = 15360  # 1024 + 2048 + 4096 + 8192
NGRP = 15  # 15 groups x 1024 cols
# u_top: levels 0..9, column order [level][pair][node]
UTOP_COLS = 8184  # 8 * 1023
UTOP_PAD = 8192


def _np_reference(contents, children, Wu, bu, Wh, bh):
    emb = None
    for i in range(DEPTH - 1, -1, -1):
        off, n = 2**i - 1, 2**i
        u = np.maximum(contents[:, off : off + n] @ Wu + bu, 0)
        if emb is None:
            emb = u
        else:
            ch = children[off : off + n] - 2 * off
            hL = emb[:, ch[:, 0]]
            hR = emb[:, ch[:, 1]]
            emb = np.maximum(
                hL @ Wh[:H] + hR @ Wh[H : 2 * H] + u @ Wh[2 * H :] + bh, 0
            )
    return emb.reshape(emb.shape[0], -1).astype(np.float32)


def _strip_pack(S):
    """Pack a [18, N] stream into the [128, ceil(N/4096)*1024] strip layout:
    wave (j, h) covers the contiguous 2048-col stream window starting at
    4096j + 2048h; its 4 strips (rows 32t..32t+17, tile cols
    [1024j + 512h : +512]) hold the window's four 512-col quarters. The 4
    strip matmuls of a wave then run concurrently in the PE sub-arrays and
    fill one [128, 2048] PSUM tile that drains with a single activation."""
    n = S.shape[1]
    out = np.zeros((128, ((n + 4095) // 4096) * 1024), dtype=S.dtype)
    for w0 in range(0, n, 2048):  # wave window
        j, h = w0 // 4096, (w0 % 4096) // 2048
        for t in range(4):
            s0 = w0 + 512 * t
            if s0 >= n:
                break
            s1 = min(s0 + 512, n)
            c0 = 1024 * j + 512 * h
            out[32 * t : 32 * t + 18, c0 : c0 + (s1 - s0)] = S[:, s0:s1]
    return out


def _prep_core_inputs(contents):
    """contents: [16, 16383, 8] f32 for one core.
    Returns dict of per-core device input arrays."""
    c4 = np.zeros((NPAIR, 128, 4096), dtype=FP8)
    big_T = np.ascontiguousarray(
        np.transpose(contents[:, 1023:16383, :], (0, 2, 1))
    )  # [16, 8, 15360]
    for p in range(NPAIR):
        S = np.empty((18, USTREAM), dtype=np.float32)
        S[0:8] = big_T[2 * p]
        S[8] = 1.0
        S[9:17] = big_T[2 * p + 1]
        S[17] = 1.0
        c4[p] = _strip_pack(S.astype(FP8))

    # u_top stream: [level][pair][node]
    tops = np.empty((18, UTOP_COLS), dtype=np.float32)
    colptr = 0
    cT = np.transpose(contents, (0, 2, 1))  # [16, 8, 16383]
    for i in range(10):
        off, n = 2**i - 1, 2**i
        for p in range(NPAIR):
            tops[0:8, colptr : colptr + n] = cT[2 * p][:, off : off + n]
            tops[8, colptr : colptr + n] = 1.0
            tops[9:17, colptr : colptr + n] = cT[2 * p + 1][:, off : off + n]
            tops[17, colptr : colptr + n] = 1.0
            colptr += n
    assert colptr == UTOP_COLS
    tb = np.zeros((18, UTOP_PAD), dtype=BF16)
    tb[:, :UTOP_COLS] = tops.astype(BF16)
    ctop = _strip_pack(tb)
    return {"c4": c4, "ctop": ctop}


def _prep_weights(Wu, bu, Wh, bh):
    wu2 = np.zeros((18, 128), dtype=np.float32)
    wu2[0:8, 0:64] = Wu
    wu2[8, 0:64] = bu
    wu2[9:17, 64:128] = Wu
    wu2[17, 64:128] = bu
    # fc_u stationary, 4-way strip layout: fp8 for the deep stream,
    # bf16 for the top-levels (ctop) stream
    wub = np.zeros((128, 128), dtype=BF16)
    wu8 = np.zeros((128, 128), dtype=FP8)
    for t in range(4):
        wub[32 * t : 32 * t + 18, :] = wu2.astype(BF16)
        wu8[32 * t : 32 * t + 18, :] = wu2.astype(FP8)

    def blockdiag(Wx):
        out = np.zeros((128, 128), dtype=np.float32)
        out[0:64, 0:64] = Wx
        out[64:128, 64:128] = Wx
        return out

    bdl = blockdiag(Wh[0:H])
    bdr = blockdiag(Wh[H : 2 * H])
    bdu = blockdiag(Wh[2 * H : 3 * H])
    whlr8 = np.concatenate([bdl, bdr], axis=1).astype(FP8)
    bh2 = np.concatenate([bh, bh]).astype(np.float32).reshape(128, 1)
    return {
        "wub": wub,
        "wu8": wu8,
        "whlr8": whlr8,
        "whlb": bdl.astype(BF16),
        "whrb": bdr.astype(BF16),
        "whub": bdu.astype(BF16),
        "bh2": bh2,
    }


def _dedup_ldweights(nc):
    """Delete an LDWEIGHTS whose signature matches the last weight load FOR
    ITS TILE POSITION (only MATMULs in between): the 128x128 PE array is 16
    independent 32x32 sub-arrays, so each tile position keeps its stationary
    operand resident until an overlapping load clobbers it. Sync info of
    deleted loads is merged into the following PE instruction."""
    n_del = 0

    def row_range(inst):
        pos = inst.tile_position
        size = inst.tile_size
        r0 = pos[0] if pos else 0
        nr = size[0] if size else 128
        return (r0, r0 + nr)

    for f in nc.m.functions:
        for bb in f.blocks:
            last_by_pos = {}  # (r0, r1) -> sig
            pending_sync = None
            out = []
            for inst in bb.instructions:
                tn = type(inst).__name__
                if str(getattr(inst, "engine", "")) == "EngineType.PE":
                    if tn == "InstLdweights":
                        a = inst.ins[0]
                        rr = row_range(inst)
                        sig = (
                            getattr(a, "memref", None),
                            getattr(a, "offset", None),
                            str(getattr(a, "ap", None)),
                            str(inst.tile_position),
                            str(inst.tile_size),
                            str(inst.perf_mode),
                            str(inst.is_transpose),
                        )
                        if last_by_pos.get(rr) == sig:
                            n_del += 1
                            si = inst.sync_info
                            if si is not None and (si.on_wait or si.on_update):
                                if pending_sync is None:
                                    pending_sync = ([], [])
                                pending_sync[0].extend(si.on_wait)
                                pending_sync[1].extend(si.on_update)
                            continue  # drop this instruction
                        # clobber any overlapping row range, then record
                        for orr in [
                            k
                            for k in last_by_pos
                            if k[0] < rr[1] and rr[0] < k[1] and k != rr
                        ]:
                            del last_by_pos[orr]
                        last_by_pos[rr] = sig
                    elif tn != "InstMatmult":
                        last_by_pos.clear()  # anything else on PE invalidates
                    if pending_sync is not None:
                        si = inst.sync_info
                        if si is None:
                            import concourse.mybir as mybir

                            inst.sync_info = mybir.SyncInfo(
                                on_wait=list(pending_sync[0]),
                                on_update=list(pending_sync[1]),
                            )
                        else:
                            si.on_wait[:0] = pending_sync[0]
                            si.on_update.extend(pending_sync[1])
                        pending_sync = None
                out.append(inst)
            assert pending_sync is None, "dangling sync from deleted trailing LDW"
            bb.instructions.clear()
            for i in out:
                bb.add_instruction(i)
    return n_del


def _split_sync_waits(nc, mybir, max_waits=1):
    """This container's walrus only accepts 1 sync-wait per instruction;
    move excess waits onto preceding same-engine NoOps."""
    for f in nc.m.functions:
        for bb in f.blocks:
            out = []
            for inst in bb.instructions:
                si = inst.sync_info
                if si is not None and len(si.on_wait) > max_waits:
                    waits = list(si.on_wait)
                    extra, keep = waits[:-max_waits], waits[-max_waits:]
                    for i in range(0, len(extra), max_waits):
                        nop = mybir.InstNoOp(
                            name=nc.get_next_instruction_name(),
                            engine=inst.engine,
                            sync_info=mybir.SyncInfo(
                                on_wait=extra[i : i + max_waits], on_update=[]
                            ),
                        )
                        out.append(nop)
                    si.on_wait = keep
                out.append(inst)
            bb.instructions.clear()
            for i in out:
                bb.add_instruction(i)


def _build_nc():
    import concourse.bass as bass
    import concourse.mybir as mybir
    from concourse.tile import TileContext

    fp32 = mybir.dt.float32
    bf16 = mybir.dt.bfloat16
    fp8 = mybir.dt.float8e4
    RELU = mybir.ActivationFunctionType.Relu
    ADD = mybir.AluOpType.add
    MAX = mybir.AluOpType.max
    DR = mybir.MatmulPerfMode.DoubleRow

    nc = bass.Bass(trn_type="TRN2", num_devices=N_CORES)
    c4_d = nc.dram_tensor("c4", [NPAIR, 128, 4096], fp8, kind="ExternalInput")
    ctop_d = nc.dram_tensor("ctop", [128, 2048], bf16, kind="ExternalInput")
    wub_d = nc.dram_tensor("wub", [128, 128], bf16, kind="ExternalInput")
    wu8_d = nc.dram_tensor("wu8", [128, 128], fp8, kind="ExternalInput")
    whlr8_d = nc.dram_tensor("whlr8", [128, 256], fp8, kind="ExternalInput")
    whlb_d = nc.dram_tensor("whlb", [128, 128], bf16, kind="ExternalInput")
    whrb_d = nc.dram_tensor("whrb", [128, 128], bf16, kind="ExternalInput")
    whub_d = nc.dram_tensor("whub", [128, 128], bf16, kind="ExternalInput")
    bh2_d = nc.dram_tensor("bh2", [128, 1], fp32, kind="ExternalInput")
    out_d = nc.dram_tensor("out", [128, NPAIR], fp32, kind="ExternalOutput")

    # greedy act-engine balance: [scalar(ACT), vector(DVE)] cumulative ns
    eng_load = [0.0, 0.0]

    with TileContext(nc) as tc:
        with (
            tc.tile_pool(name="wpool", bufs=1) as wpool,
            tc.tile_pool(name="c4pool", bufs=3) as c4pool,
            tc.tile_pool(name="uspool", bufs=4) as uspool,
            tc.tile_pool(name="e12pool", bufs=2) as e12pool,
            tc.tile_pool(name="e11pool", bufs=2) as e11pool,
            tc.tile_pool(name="shpool", bufs=1) as shpool,
            tc.tile_pool(name="pspool", bufs=4, space="PSUM") as pspool,
        ):
            whlr_sb = wpool.tile([128, 256], fp8, tag="whlr")
            bh_sb = wpool.tile([128, 1], fp32, tag="bh")
            wub_sb = wpool.tile([128, 128], bf16, tag="wub")
            wu8_sb = wpool.tile([128, 128], fp8, tag="wu8")
            whlb_sb = wpool.tile([128, 128], bf16, tag="whlb")
            whrb_sb = wpool.tile([128, 128], bf16, tag="whrb")
            whub_sb = wpool.tile([128, 128], bf16, tag="whub")
            ctop_sb = wpool.tile([128, 2048], bf16, tag="ctop")
            utop = wpool.tile([128, UTOP_PAD], bf16, tag="utop")
            # only the fc_u weights gate the first wave; everything else is
            # issued after pair 0's chunks (Sync DMA issue is ~600ns each)
            nc.sync.dma_start(wu8_sb[:], wu8_d.ap())

            whlr_v = whlr_sb[:, 0:256].rearrange("p (two m) -> p two m", two=2)

            def act_relu(dst_ap, src_ap, bias, ncols):
                """relu(src + bias) -> dst on the act engine with the least
                estimated accumulated load."""
                cost = (230.0 + 0.833 * ncols, 147.0 + 1.042 * ncols)
                e = 0 if eng_load[0] + cost[0] <= eng_load[1] + cost[1] else 1
                eng_load[e] += cost[e]
                if e == 0:
                    if bias is None:
                        nc.scalar.activation(dst_ap, src_ap, RELU)
                    else:
                        nc.scalar.activation(dst_ap, src_ap, RELU, bias=bias)
                else:
                    if bias is None:
                        nc.vector.tensor_scalar(dst_ap, src_ap, 0.0, None, MAX)
                    else:
                        nc.vector.tensor_scalar(dst_ap, src_ap, bias, 0.0, ADD, MAX)

            def u_phase(p, chunks, ustrb, ustr8):
                """fc_u for the deep stream of one pair (bf16, strip-tiled).
                Each wave = 4 matmuls at different tile_positions, run
                CONCURRENTLY by the PE's 32-row sub-arrays (~4x throughput),
                filling the four 512-col quarters of ONE [128, 2048] PSUM
                tile = one contiguous stream window = one activation.
                Inner-node u (cols < 7168) lands in bf16 for the U-term;
                leaf u lands in fp8 for the level-12 fused L+R."""
                for w in range(8):
                    s0 = 2048 * w
                    nstrip = min(4, (USTREAM - s0 + 511) // 512)
                    ch, cb = chunks[w // 2]
                    hc = cb + 512 * (w % 2)
                    # wave of up-to-4 concurrent strip matmuls filling two
                    # [128, 1024] psum tiles (strips 0-1 / 2-3)
                    pss = [
                        pspool.tile(
                            [128, 1024], fp32, tag="ps", name=f"psu{p}_{w}_{k}"
                        )
                        for k in range((nstrip + 1) // 2)
                    ]
                    for t in range(nstrip):
                        nc.tensor.matmul(
                            pss[t // 2][:, 512 * (t % 2) : 512 * (t % 2 + 1)],
                            wu8_sb[32 * t : 32 * t + 18, :],
                            ch[32 * t : 32 * t + 18, hc : hc + 512],
                            start=True,
                            stop=True,
                            tile_position=(32 * t, 0),
                        )
                    for k in range((nstrip + 1) // 2):
                        a0 = s0 + 1024 * k
                        a1 = min(a0 + 1024, s0 + 512 * nstrip)
                        if a0 < ULEAF:
                            dstt, base = ustrb, 0
                        else:
                            dstt, base = ustr8, ULEAF
                        act_relu(
                            dstt[:, a0 - base : a1 - base],
                            pss[k][:, 0 : a1 - a0],
                            None,
                            a1 - a0,
                        )

            def levels_deep(p, ustrb, ustr8, emb10sh):
                """fc_h levels 12..10 for one pair: fused L+R via fp8
                DoubleRow, U-term in bf16, 2048-col supergroups -> one act
                per supergroup and 2 weight switches."""
                emb12 = e12pool.tile([128, 4096], fp8, tag="e12")
                emb11 = e11pool.tile([128, 2048], fp8, tag="e11")
                for i, ubase, prev, dst, dst_base in (
                    (12, UB12, ustr8, emb12, 0),
                    (11, UB11, emb12, emb11, 0),
                    (10, UB10, emb11, emb10sh, 1024 * p),
                ):
                    m = 2**i
                    groups = list(range(0, m, 1024))
                    for g0 in range(0, len(groups), 2):
                        grp = groups[g0 : g0 + 2]
                        pss = [
                            pspool.tile(
                                [128, 1024], fp32, tag="ps", name=f"psl{p}_{i}_{s0}"
                            )
                            for s0 in grp
                        ]
                        for ci, s0 in enumerate(grp):
                            w = min(1024, m - s0)
                            for h0 in range(0, w, 512):
                                j0 = s0 + h0
                                mv = prev[:, 2 * j0 : 2 * j0 + 1024].rearrange(
                                    "p (n two) -> p two n", two=2
                                )
                                nc.tensor.matmul(
                                    pss[ci][:, h0 : h0 + 512],
                                    whlr_v,
                                    mv,
                                    start=True,
                                    stop=False,
                                    perf_mode=DR,
                                )
                        for ci, s0 in enumerate(grp):
                            w = min(1024, m - s0)
                            for h0 in range(0, w, 512):
                                j0 = s0 + h0
                                nc.tensor.matmul(
                                    pss[ci][:, h0 : h0 + 512],
                                    whub_sb[:],
                                    ustrb[:, ubase + j0 : ubase + j0 + 512],
                                    start=False,
                                    stop=True,
                                )
                        for ci, s0 in enumerate(grp):
                            w = min(1024, m - s0)
                            act_relu(
                                dst[:, dst_base + s0 : dst_base + s0 + w],
                                pss[ci][:, 0:w],
                                bh_sb[:],
                                w,
                            )

            # ---- pairs, software-pipelined: u(p) emitted before levels(p-1)
            # so independent u work fills the level chains' act stalls ----
            emb10sh = shpool.tile([128, 8192], fp8, tag="e10")
            ustrbs = [None] * NPAIR
            ustr8s = [None] * NPAIR
            for p in range(NPAIR):
                chunks = []
                if p == 0:
                    # chunked first pair so the first wave starts ~1us in
                    for j in range(4):
                        ch = c4pool.tile(
                            [128, 1024], fp8, tag="c4", name=f"c4_{p}_{j}"
                        )
                        nc.sync.dma_start(
                            ch[:], c4_d.ap()[p][:, 1024 * j : 1024 * (j + 1)]
                        )
                        chunks.append((ch, 0))
                else:
                    # one DMA per pair: issue cost on the Sync engine is the
                    # startup bottleneck, not transfer bandwidth
                    whole = c4pool.tile([128, 4096], fp8, tag="c4w", name=f"c4w{p}")
                    nc.sync.dma_start(whole[:], c4_d.ap()[p])
                    chunks = [(whole, 1024 * j) for j in range(4)]
                ustrbs[p] = uspool.tile([128, 7168], bf16, tag="usb", name=f"ustrb{p}")
                ustr8s[p] = uspool.tile([128, 8192], fp8, tag="us8", name=f"ustr8{p}")
                u_phase(p, chunks, ustrbs[p], ustr8s[p])
                if p == 0:
                    # remaining weights + tail inputs, after pair 0's chunks
                    nc.sync.dma_start(whlr_sb[:], whlr8_d.ap())
                    nc.sync.dma_start(whub_sb[:], whub_d.ap())
                    nc.sync.dma_start(bh_sb[:], bh2_d.ap())
                    nc.sync.dma_start(wub_sb[:], wub_d.ap())
                    nc.sync.dma_start(whlb_sb[:], whlb_d.ap())
                    nc.sync.dma_start(whrb_sb[:], whrb_d.ap())
                    nc.sync.dma_start(ctop_sb[:], ctop_d.ap())
                if p > 0:
                    levels_deep(p - 1, ustrbs[p - 1], ustr8s[p - 1], emb10sh)
            levels_deep(NPAIR - 1, ustrbs[NPAIR - 1], ustr8s[NPAIR - 1], emb10sh)

            # ---- u for levels 0..9 (strip waves), emitted late as filler ----
            for w in range(4):
                cc = 1024 * (w // 2) + 512 * (w % 2)
                pss = [
                    pspool.tile([128, 1024], fp32, tag="ps", name=f"psut{w}_{k}")
                    for k in range(2)
                ]
                for t in range(4):
                    nc.tensor.matmul(
                        pss[t // 2][:, 512 * (t % 2) : 512 * (t % 2 + 1)],
                        wub_sb[32 * t : 32 * t + 18, :],
                        ctop_sb[32 * t : 32 * t + 18, cc : cc + 512],
                        start=True,
                        stop=True,
                        tile_position=(32 * t, 0),
                    )
                for k in range(2):
                    act_relu(
                        utop[:, 2048 * w + 1024 * k : 2048 * w + 1024 * (k + 1)],
                        pss[k][:, 0:1024],
                        None,
                        1024,
                    )

            # ---- levels 9..1 in two independent 4-pair chains ----
            # Levels 9..8 use the fused fp8 L+R (inputs emb10sh/esh9 are fp8);
            # levels 7..1 are bf16 (fp8 noise near the root does not average).
            eshs = [{}, {}]
            for i in range(9, 0, -1):
                m4 = 4 * (2**i)
                b8 = 8 * (2**i - 1)
                fused = i >= 8
                for X in range(2):
                    prev = emb10sh if i == 9 else eshs[X][i + 1]
                    pb = 4096 * X if i == 9 else 0
                    cur = wpool.tile(
                        [128, m4], fp8 if i == 9 else bf16, tag=f"esh{X}_{i}"
                    )
                    eshs[X][i] = cur
                    for s0 in range(0, m4, 1024):
                        w = min(1024, m4 - s0)
                        ps = pspool.tile(
                            [128, 1024], fp32, tag="ps", name=f"pst{X}_{i}_{s0}"
                        )
                        if fused:
                            for h0 in range(0, w, 512):
                                n = min(512, w - h0)
                                j0 = s0 + h0
                                mv = prev[
                                    :, pb + 2 * j0 : pb + 2 * j0 + 2 * n
                                ].rearrange("p (n two) -> p two n", two=2)
                                nc.tensor.matmul(
                                    ps[:, h0 : h0 + n],
                                    whlr_v,
                                    mv,
                                    start=True,
                                    stop=False,
                                    perf_mode=DR,
                                )
                            for h0 in range(0, w, 512):
                                n = min(512, w - h0)
                                j0 = s0 + h0
                                nc.tensor.matmul(
                                    ps[:, h0 : h0 + n],
                                    whub_sb[:],
                                    utop[:, b8 + m4 * X + j0 : b8 + m4 * X + j0 + n],
                                    start=False,
                                    stop=True,
                                )
                        else:
                            for w_sb, kind in (
                                (whlb_sb, "L"),
                                (whrb_sb, "R"),
                                (whub_sb, "U"),
                            ):
                                for h0 in range(0, w, 512):
                                    n = min(512, w - h0)
                                    j0 = s0 + h0
                                    if kind == "L":
                                        mv = prev[
                                            :, pb + 2 * j0 : pb + 2 * j0 + 2 * n : 2
                                        ]
                                    elif kind == "R":
                                        mv = prev[
                                            :,
                                            pb + 2 * j0 + 1 : pb + 2 * j0 + 2 * n : 2,
                                        ]
                                    else:
                                        mv = utop[
                                            :, b8 + m4 * X + j0 : b8 + m4 * X + j0 + n
                                        ]
                                    nc.tensor.matmul(
                                        ps[:, h0 : h0 + n],
                                        w_sb[:],
                                        mv,
                                        start=(kind == "L"),
                                        stop=(kind == "U"),
                                    )
                        act_relu(cur[:, s0 : s0 + w], ps[:, 0:w], bh_sb[:], w)

            # ---- level 0: roots, one per chain ----
            roots = wpool.tile([128, NPAIR], fp32, tag="roots")
            for X in range(2):
                ps = pspool.tile([128, 1024], fp32, tag="ps", name=f"psroot{X}")
                o = ps[:, 0:4]
                e1 = eshs[X][1]
                nc.tensor.matmul(o, whlb_sb[:], e1[:, 0:8:2], start=True, stop=False)
                nc.tensor.matmul(o, whrb_sb[:], e1[:, 1:8:2], start=False, stop=False)
                nc.tensor.matmul(
                    o, whub_sb[:], utop[:, 4 * X : 4 * X + 4], start=False, stop=True
                )
                nc.scalar.activation(
                    roots[:, 4 * X : 4 * X + 4], o, RELU, bias=bh_sb[:]
                )
            nc.sync.dma_start(out_d.ap(), roots[:])

    _dedup_ldweights(nc)
    _split_sync_waits(nc, mybir)
    return nc


_NC_CACHE = None
LAST_RESULTS = None


def kernel(contents, children, Wu, bu, Wh, bh):
    global _NC_CACHE, LAST_RESULTS
    contents = np.asarray(contents, dtype=np.float32)
    children = np.asarray(children)
    Wu = np.asarray(Wu, dtype=np.float32)
    bu = np.asarray(bu, dtype=np.float32)
    Wh = np.asarray(Wh, dtype=np.float32)
    bh = np.asarray(bh, dtype=np.float32)

    regular = (
        contents.shape == (B, N_NODES, F)
        and children.shape == (N_INNER, 2)
        and np.array_equal(
            np.asarray(children, dtype=np.int64).ravel(), np.arange(N_INNER * 2)
        )
    )
    if not regular:
        # Safety net for non-arange children: exact numpy fallback.
        return _np_reference(contents, children, Wu, bu, Wh, bh)

    from concourse.bass_utils import run_bass_kernel_spmd

    if _NC_CACHE is None:
        _NC_CACHE = _build_nc()
    nc = _NC_CACHE

    wts = _prep_weights(Wu, bu, Wh, bh)
    in_maps = []
    for k in range(N_CORES):
        m = _prep_core_inputs(contents[JPC * k : JPC * (k + 1)])
        m.update(wts)
        in_maps.append(m)

    res = run_bass_kernel_spmd(
        nc,
        in_maps,
        core_ids=list(range(N_CORES)),
        trace=bool(os.environ.get("BASS_TRACE")),
    )
    LAST_RESULTS = res

    out = np.empty((B, H), dtype=np.float32)
    for k in range(N_CORES):
        r = res.results[k]["out"].reshape(2, 64, NPAIR)  # [half, h, pair]
        out[JPC * k : JPC * (k + 1)] = np.transpose(r, (2, 0, 1)).reshape(JPC, H)
    return out


# revision 40
# speedup vs baseline: 1.0306x; 1.0135x over previous
"""Trainium2 Bass kernel for GRNNTransformSimple (bottom-up binary-tree GRNN).

Computation (per jet): heap-layout complete binary tree, DEPTH=14.
  u_k   = relu(contents_k @ Wu + bu)                         (all nodes)
  emb_k = u_k                                                (leaves)
  emb_k = relu(hL @ Wh[:64] + hR @ Wh[64:128] + u_k @ Wh[128:] + bh)  (inner)
Output: root emb, [B, 64].

Mapping (8 NeuronCores, data-parallel over B=128 jets, 16 jets/core):
 - 2 jets packed per 128 SBUF partitions (jet A on partitions 0-63, jet B on
   64-127) with block-diagonal weights -> all engines run 128 partitions wide.
 - The PE streams exactly 1 output column per cycle in every mode
   (measured: bf16 = fp8 = fp8-DoubleRow = 216ns per 512-col matmul), so
   the only matmul-count lever is fusing the L and R contributions into a
   single pass: fp8e4m3 DoubleRow with stationary planes (WhL_bd, WhR_bd)
   and moving planes (hL, hR) = the even/odd column interleave of the
   child level. Applied at the deep levels (12..10, 87% of fc_h work).
   Deep-level fp8 quantization noise averages out on the way up the tree
   (measured: rel_rms 4.1e-3 vs 4.1e-3 pure-bf16 on the reference inputs);
   fp8 on the top levels does NOT average and is left in bf16.
 - fc_u biases folded into the matmul via a constant-one input row (K=18),
   4-way strip-tiled (tile_position) since K=18 << 128. Each 2048-col
   stream window is issued as a WAVE of 4 consecutive strip matmuls at
   different tile positions: the PE's 16x 32x32 sub-arrays execute them
   CONCURRENTLY (measured 53ns/matmul vs 216 serial, ~4x), so fc_u drops
   from 51us to ~13us of PE time. The deep fc_u stream (c4 + its weights)
   is fp8 (halves the startup DMA and SBUF reads; same PE speed).
 - fc_h U-term stays bf16 (same pass count as fp8, cheap fast-weight-load).
 - Levels 9..0 are batched across pairs, split into two independent 4-pair
   chains so one chain's matmuls fill the other's activation stalls;
   levels 9..8 use the fused fp8 L+R, 7..0 bf16 (top-level u stays bf16).
 - The "irregular" child gather is regular for arange children: children of
   level-i node j are nodes 2j, 2j+1 of level i+1, i.e. stride-2 column
   slices (the fp8 DoubleRow plane view) of the level-(i+1) embedding.
 - Pair 0's c4 is DMA'd in 128KB column chunks so the first wave starts
   ~1us in; later pairs use one DMA each (Sync-engine issue cost, ~600ns
   per DMA, dominates over transfer bandwidth). The u phase for levels
   0..9 (utop) is emitted last to fill tail stalls.
 - relu activations are split between the Scalar and Vector engines by a
   greedy balance on estimated cost; with the PE tricks above the act
   engines are co-critical with the PE (~110us busy each).
"""

import os
import sys

sys.path.insert(0, "/opt/trn_rl_repo")

import ml_dtypes
import numpy as np

DEPTH = 14
B = 128
F = 8
H = 64
N_NODES = 2**DEPTH - 1  # 16383
N_INNER = 2 ** (DEPTH - 1) - 1  # 8191
N_CORES = 8
JPC = 16  # jets per core
NPAIR = 8  # jet pairs per core

BF16 = ml_dtypes.bfloat16
FP8 = ml_dtypes.float8_e4m3fn

# u_stream layout per pair (columns): levels 10,11,12 inner nodes in heap
# order, then all leaves in heap order.
UB10, UB11, UB12 = 0, 1024, 3072  # level bases inside u_stream
ULEAF = 7168
USTREAM = 15360  # 1024 + 2048 + 4096 + 8192
NGRP = 15  # 15 groups x 1024 cols
# u_top: levels 0..9, column order [level][pair][node]
UTOP_COLS = 8184  # 8 * 1023
UTOP_PAD = 8192


def _np_reference(contents, children, Wu, bu, Wh, bh):
    emb = None
    for i in range(DEPTH - 1, -1, -1):
        off, n = 2**i - 1, 2**i
        u = np.maximum(contents[:, off : off + n] @ Wu + bu, 0)
        if emb is None:
            emb = u
        else:
            ch = children[off : off + n] - 2 * off
            hL = emb[:, ch[:, 0]]
            hR = emb[:, ch[:, 1]]
            emb = np.maximum(
                hL @ Wh[:H] + hR @ Wh[H : 2 * H] + u @ Wh[2 * H :] + bh, 0
            )
    return emb.reshape(emb.shape[0], -1).astype(np.float32)


def _strip_pack(S):
    """Pack a [18, N] stream into the [128, ceil(N/4096)*1024] strip layout:
    wave (j, h) covers the contiguous 2048-col stream window starting at
    4096j + 2048h; its 4 strips (rows 32t..32t+17, tile cols
    [1024j + 512h : +512]) hold the window's four 512-col quarters. The 4
    strip matmuls of a wave then run concurrently in the PE sub-arrays and
    fill one [128, 2048] PSUM tile that drains with a single activation."""
    n = S.shape[1]
    out = np.zeros((128, ((n + 4095) // 4096) * 1024), dtype=S.dtype)
    for w0 in range(0, n, 2048):  # wave window
        j, h = w0 // 4096, (w0 % 4096) // 2048
        for t in range(4):
            s0 = w0 + 512 * t
            if s0 >= n:
                break
            s1 = min(s0 + 512, n)
            c0 = 1024 * j + 512 * h
            out[32 * t : 32 * t + 18, c0 : c0 + (s1 - s0)] = S[:, s0:s1]
    return out


def _prep_core_inputs(contents):
    """contents: [16, 16383, 8] f32 for one core.
    Returns dict of per-core device input arrays."""
    c4 = np.zeros((NPAIR, 128, 4096), dtype=FP8)
    big_T = np.ascontiguousarray(
        np.transpose(contents[:, 1023:16383, :], (0, 2, 1))
    )  # [16, 8, 15360]
    for p in range(NPAIR):
        S = np.empty((18, USTREAM), dtype=np.float32)
        S[0:8] = big_T[2 * p]
        S[8] = 1.0
        S[9:17] = big_T[2 * p + 1]
        S[17] = 1.0
        c4[p] = _strip_pack(S.astype(FP8))

    # u_top stream: [level][pair][node]
    tops = np.empty((18, UTOP_COLS), dtype=np.float32)
    colptr = 0
    cT = np.transpose(contents, (0, 2, 1))  # [16, 8, 16383]
    for i in range(10):
        off, n = 2**i - 1, 2**i
        for p in range(NPAIR):
            tops[0:8, colptr : colptr + n] = cT[2 * p][:, off : off + n]
            tops[8, colptr : colptr + n] = 1.0
            tops[9:17, colptr : colptr + n] = cT[2 * p + 1][:, off : off + n]
            tops[17, colptr : colptr + n] = 1.0
            colptr += n
    assert colptr == UTOP_COLS
    tb = np.zeros((18, UTOP_PAD), dtype=BF16)
    tb[:, :UTOP_COLS] = tops.astype(BF16)
    ctop = _strip_pack(tb)
    return {"c4": c4, "ctop": ctop}


def _prep_weights(Wu, bu, Wh, bh):
    wu2 = np.zeros((18, 128), dtype=np.float32)
    wu2[0:8, 0:64] = Wu
    wu2[8, 0:64] = bu
    wu2[9:17, 64:128] = Wu
    wu2[17, 64:128] = bu
    # fc_u stationary, 4-way strip layout: fp8 for the deep stream,
    # bf16 for the top-levels (ctop) stream
    wub = np.zeros((128, 128), dtype=BF16)
    wu8 = np.zeros((128, 128), dtype=FP8)
    for t in range(4):
        wub[32 * t : 32 * t + 18, :] = wu2.astype(BF16)
        wu8[32 * t : 32 * t + 18, :] = wu2.astype(FP8)

    def blockdiag(Wx):
        out = np.zeros((128, 128), dtype=np.float32)
        out[0:64, 0:64] = Wx
        out[64:128, 64:128] = Wx
        return out

    bdl = blockdiag(Wh[0:H])
    bdr = blockdiag(Wh[H : 2 * H])
    bdu = blockdiag(Wh[2 * H : 3 * H])
    whlr8 = np.concatenate([bdl, bdr], axis=1).astype(FP8)
    bh2 = np.concatenate([bh, bh]).astype(np.float32).reshape(128, 1)
    return {
        "wub": wub,
        "wu8": wu8,
        "whlr8": whlr8,
        "whlb": bdl.astype(BF16),
        "whrb": bdr.astype(BF16),
        "whub": bdu.astype(BF16),
        "bh2": bh2,
    }


def _dedup_ldweights(nc):
    """Delete an LDWEIGHTS whose signature matches the last weight load FOR
    ITS TILE POSITION (only MATMULs in between): the 128x128 PE array is 16
    independent 32x32 sub-arrays, so each tile position keeps its stationary
    operand resident until an overlapping load clobbers it. Sync info of
    deleted loads is merged into the following PE instruction."""
    n_del = 0

    def row_range(inst):
        pos = inst.tile_position
        size = inst.tile_size
        r0 = pos[0] if pos else 0
        nr = size[0] if size else 128
        return (r0, r0 + nr)

    for f in nc.m.functions:
        for bb in f.blocks:
            last_by_pos = {}  # (r0, r1) -> sig
            pending_sync = None
            out = []
            for inst in bb.instructions:
                tn = type(inst).__name__
                if str(getattr(inst, "engine", "")) == "EngineType.PE":
                    if tn == "InstLdweights":
                        a = inst.ins[0]
                        rr = row_range(inst)
                        sig = (
                            getattr(a, "memref", None),
                            getattr(a, "offset", None),
                            str(getattr(a, "ap", None)),
                            str(inst.tile_position),
                            str(inst.tile_size),
                            str(inst.perf_mode),
                            str(inst.is_transpose),
                        )
                        if last_by_pos.get(rr) == sig:
                            n_del += 1
                            si = inst.sync_info
                            if si is not None and (si.on_wait or si.on_update):
                                if pending_sync is None:
                                    pending_sync = ([], [])
                                pending_sync[0].extend(si.on_wait)
                                pending_sync[1].extend(si.on_update)
                            continue  # drop this instruction
                        # clobber any overlapping row range, then record
                        for orr in [
                            k
                            for k in last_by_pos
                            if k[0] < rr[1] and rr[0] < k[1] and k != rr
                        ]:
                            del last_by_pos[orr]
                        last_by_pos[rr] = sig
                    elif tn != "InstMatmult":
                        last_by_pos.clear()  # anything else on PE invalidates
                    if pending_sync is not None:
                        si = inst.sync_info
                        if si is None:
                            import concourse.mybir as mybir

                            inst.sync_info = mybir.SyncInfo(
                                on_wait=list(pending_sync[0]),
                                on_update=list(pending_sync[1]),
                            )
                        else:
                            si.on_wait[:0] = pending_sync[0]
                            si.on_update.extend(pending_sync[1])
                        pending_sync = None
                out.append(inst)
            assert pending_sync is None, "dangling sync from deleted trailing LDW"
            bb.instructions.clear()
            for i in out:
                bb.add_instruction(i)
    return n_del


def _split_sync_waits(nc, mybir, max_waits=1):
    """This container's walrus only accepts 1 sync-wait per instruction;
    move excess waits onto preceding same-engine NoOps."""
    for f in nc.m.functions:
        for bb in f.blocks:
            out = []
            for inst in bb.instructions:
                si = inst.sync_info
                if si is not None and len(si.on_wait) > max_waits:
                    waits = list(si.on_wait)
                    extra, keep = waits[:-max_waits], waits[-max_waits:]
                    for i in range(0, len(extra), max_waits):
                        nop = mybir.InstNoOp(
                            name=nc.get_next_instruction_name(),
                            engine=inst.engine,
                            sync_info=mybir.SyncInfo(
                                on_wait=extra[i : i + max_waits], on_update=[]
                            ),
                        )
                        out.append(nop)
                    si.on_wait = keep
                out.append(inst)
            bb.instructions.clear()
            for i in out:
                bb.add_instruction(i)


def _build_nc():
    import concourse.bass as bass
    import concourse.mybir as mybir
    from concourse.tile import TileContext

    fp32 = mybir.dt.float32
    bf16 = mybir.dt.bfloat16
    fp8 = mybir.dt.float8e4
    RELU = mybir.ActivationFunctionType.Relu
    ADD = mybir.AluOpType.add
    MAX = mybir.AluOpType.max
    DR = mybir.MatmulPerfMode.DoubleRow

    nc = bass.Bass(trn_type="TRN2", num_devices=N_CORES)
    c4_d = nc.dram_tensor("c4", [NPAIR, 128, 4096], fp8, kind="ExternalInput")
    ctop_d = nc.dram_tensor("ctop", [128, 2048], bf16, kind="ExternalInput")
    wub_d = nc.dram_tensor("wub", [128, 128], bf16, kind="ExternalInput")
    wu8_d = nc.dram_tensor("wu8", [128, 128], fp8, kind="ExternalInput")
    whlr8_d = nc.dram_tensor("whlr8", [128, 256], fp8, kind="ExternalInput")
    whlb_d = nc.dram_tensor("whlb", [128, 128], bf16, kind="ExternalInput")
    whrb_d = nc.dram_tensor("whrb", [128, 128], bf16, kind="ExternalInput")
    whub_d = nc.dram_tensor("whub", [128, 128], bf16, kind="ExternalInput")
    bh2_d = nc.dram_tensor("bh2", [128, 1], fp32, kind="ExternalInput")
    out_d = nc.dram_tensor("out", [128, NPAIR], fp32, kind="ExternalOutput")

    # greedy act-engine balance: [scalar(ACT), vector(DVE)] cumulative ns
    eng_load = [0.0, 0.0]

    with TileContext(nc) as tc:
        with (
            tc.tile_pool(name="wpool", bufs=1) as wpool,
            tc.tile_pool(name="c4pool", bufs=3) as c4pool,
            tc.tile_pool(name="uspool", bufs=4) as uspool,
            tc.tile_pool(name="e12pool", bufs=2) as e12pool,
            tc.tile_pool(name="e11pool", bufs=2) as e11pool,
            tc.tile_pool(name="shpool", bufs=1) as shpool,
            tc.tile_pool(name="pspool", bufs=4, space="PSUM") as pspool,
        ):
            whlr_sb = wpool.tile([128, 256], fp8, tag="whlr")
            bh_sb = wpool.tile([128, 1], fp32, tag="bh")
            wub_sb = wpool.tile([128, 128], bf16, tag="wub")
            wu8_sb = wpool.tile([128, 128], fp8, tag="wu8")
            whlb_sb = wpool.tile([128, 128], bf16, tag="whlb")
            whrb_sb = wpool.tile([128, 128], bf16, tag="whrb")
            whub_sb = wpool.tile([128, 128], bf16, tag="whub")
            ctop_sb = wpool.tile([128, 2048], bf16, tag="ctop")
            utop = wpool.tile([128, UTOP_PAD], bf16, tag="utop")
            # only the fc_u weights gate the first wave; everything else is
            # issued after pair 0's chunks (Sync DMA issue is ~600ns each)
            nc.sync.dma_start(wu8_sb[:], wu8_d.ap())

            whlr_v = whlr_sb[:, 0:256].rearrange("p (two m) -> p two m", two=2)

            def act_relu(dst_ap, src_ap, bias, ncols):
                """relu(src + bias) -> dst on the act engine with the least
                estimated accumulated load."""
                cost = (230.0 + 0.833 * ncols, 147.0 + 1.042 * ncols)
                e = 0 if eng_load[0] + cost[0] <= eng_load[1] + cost[1] else 1
                eng_load[e] += cost[e]
                if e == 0:
                    if bias is None:
                        nc.scalar.activation(dst_ap, src_ap, RELU)
                    else:
                        nc.scalar.activation(dst_ap, src_ap, RELU, bias=bias)
                else:
                    if bias is None:
                        nc.vector.tensor_scalar(dst_ap, src_ap, 0.0, None, MAX)
                    else:
                        nc.vector.tensor_scalar(dst_ap, src_ap, bias, 0.0, ADD, MAX)

            def u_phase(p, chunks, ustrb, ustr8):
                """fc_u for the deep stream of one pair (bf16, strip-tiled).
                Each wave = 4 matmuls at different tile_positions, run
                CONCURRENTLY by the PE's 32-row sub-arrays (~4x throughput),
                filling the four 512-col quarters of ONE [128, 2048] PSUM
                tile = one contiguous stream window = one activation.
                Inner-node u (cols < 7168) lands in bf16 for the U-term;
                leaf u lands in fp8 for the level-12 fused L+R."""
                for w in range(8):
                    s0 = 2048 * w
                    nstrip = min(4, (USTREAM - s0 + 511) // 512)
                    ch, cb = chunks[w // 2]
                    hc = cb + 512 * (w % 2)
                    # wave of up-to-4 concurrent strip matmuls filling two
                    # [128, 1024] psum tiles (strips 0-1 / 2-3)
                    pss = [
                        pspool.tile(
                            [128, 1024], fp32, tag="ps", name=f"psu{p}_{w}_{k}"
                        )
                        for k in range((nstrip + 1) // 2)
                    ]
                    for t in range(nstrip):
                        nc.tensor.matmul(
                            pss[t // 2][:, 512 * (t % 2) : 512 * (t % 2 + 1)],
                            wu8_sb[32 * t : 32 * t + 18, :],
                            ch[32 * t : 32 * t + 18, hc : hc + 512],
                            start=True,
                            stop=True,
                            tile_position=(32 * t, 0),
                        )
                    for k in range((nstrip + 1) // 2):
                        a0 = s0 + 1024 * k
                        a1 = min(a0 + 1024, s0 + 512 * nstrip)
                        if a0 < ULEAF:
                            dstt, base = ustrb, 0
                        else:
                            dstt, base = ustr8, ULEAF
                        act_relu(
                            dstt[:, a0 - base : a1 - base],
                            pss[k][:, 0 : a1 - a0],
                            None,
                            a1 - a0,
                        )

            def levels_deep(p, ustrb, ustr8, emb10sh):
                """fc_h levels 12..10 for one pair: fused L+R via fp8
                DoubleRow, U-term in bf16, 2048-col supergroups -> one act
                per supergroup and 2 weight switches."""
                emb12 = e12pool.tile([128, 4096], fp8, tag="e12")
                emb11 = e11pool.tile([128, 2048], fp8, tag="e11")
                for i, ubase, prev, dst, dst_base in (
                    (12, UB12, ustr8, emb12, 0),
                    (11, UB11, emb12, emb11, 0),
                    (10, UB10, emb11, emb10sh, 1024 * p),
                ):
                    m = 2**i
                    groups = list(range(0, m, 1024))
                    for g0 in range(0, len(groups), 2):
                        grp = groups[g0 : g0 + 2]
                        pss = [
                            pspool.tile(
                                [128, 1024], fp32, tag="ps", name=f"psl{p}_{i}_{s0}"
                            )
                            for s0 in grp
                        ]
                        for ci, s0 in enumerate(grp):
                            w = min(1024, m - s0)
                            for h0 in range(0, w, 512):
                                j0 = s0 + h0
                                mv = prev[:, 2 * j0 : 2 * j0 + 1024].rearrange(
                                    "p (n two) -> p two n", two=2
                                )
                                nc.tensor.matmul(
                                    pss[ci][:, h0 : h0 + 512],
                                    whlr_v,
                                    mv,
                                    start=True,
                                    stop=False,
                                    perf_mode=DR,
                                )
                        for ci, s0 in enumerate(grp):
                            w = min(1024, m - s0)
                            for h0 in range(0, w, 512):
                                j0 = s0 + h0
                                nc.tensor.matmul(
                                    pss[ci][:, h0 : h0 + 512],
                                    whub_sb[:],
                                    ustrb[:, ubase + j0 : ubase + j0 + 512],
                                    start=False,
                                    stop=True,
                                )
                        for ci, s0 in enumerate(grp):
                            w = min(1024, m - s0)
                            act_relu(
                                dst[:, dst_base + s0 : dst_base + s0 + w],
                                pss[ci][:, 0:w],
                                bh_sb[:],
                                w,
                            )

            # ---- pairs, software-pipelined: u(p) emitted before levels(p-1)
            # so independent u work fills the level chains' act stalls ----
            emb10sh = shpool.tile([128, 8192], fp8, tag="e10")
            ustrbs = [None] * NPAIR
            ustr8s = [None] * NPAIR
            for p in range(NPAIR):
                chunks = []
                if p == 0:
                    # chunked first pair so the first wave starts ~1us in
                    for j in range(4):
                        ch = c4pool.tile(
                            [128, 1024], fp8, tag="c4", name=f"c4_{p}_{j}"
                        )
                        nc.sync.dma_start(
                            ch[:], c4_d.ap()[p][:, 1024 * j : 1024 * (j + 1)]
                        )
                        chunks.append((ch, 0))
                else:
                    # one DMA per pair: issue cost on the Sync engine is the
                    # startup bottleneck, not transfer bandwidth
                    whole = c4pool.tile([128, 4096], fp8, tag="c4w", name=f"c4w{p}")
                    nc.sync.dma_start(whole[:], c4_d.ap()[p])
                    chunks = [(whole, 1024 * j) for j in range(4)]
                ustrbs[p] = uspool.tile([128, 7168], bf16, tag="usb", name=f"ustrb{p}")
                ustr8s[p] = uspool.tile([128, 8192], fp8, tag="us8", name=f"ustr8{p}")
                u_phase(p, chunks, ustrbs[p], ustr8s[p])
                if p == 0:
                    # remaining weights + tail inputs, after pair 0's chunks
                    nc.sync.dma_start(whlr_sb[:], whlr8_d.ap())
                    nc.sync.dma_start(whub_sb[:], whub_d.ap())
                    nc.sync.dma_start(bh_sb[:], bh2_d.ap())
                    nc.sync.dma_start(wub_sb[:], wub_d.ap())
                    nc.sync.dma_start(whlb_sb[:], whlb_d.ap())
                    nc.sync.dma_start(whrb_sb[:], whrb_d.ap())
                    nc.sync.dma_start(ctop_sb[:], ctop_d.ap())
                if p > 0:
                    levels_deep(p - 1, ustrbs[p - 1], ustr8s[p - 1], emb10sh)

            # ---- u for levels 0..9 (strip waves): emitted before the last
            # deep pair so its activations drain during the deep phase and
            # the top chains can start the moment emb10sh completes ----
            for w in range(4):
                cc = 1024 * (w // 2) + 512 * (w % 2)
                pss = [
                    pspool.tile([128, 1024], fp32, tag="ps", name=f"psut{w}_{k}")
                    for k in range(2)
                ]
                for t in range(4):
                    nc.tensor.matmul(
                        pss[t // 2][:, 512 * (t % 2) : 512 * (t % 2 + 1)],
                        wub_sb[32 * t : 32 * t + 18, :],
                        ctop_sb[32 * t : 32 * t + 18, cc : cc + 512],
                        start=True,
                        stop=True,
                        tile_position=(32 * t, 0),
                    )
                for k in range(2):
                    act_relu(
                        utop[:, 2048 * w + 1024 * k : 2048 * w + 1024 * (k + 1)],
                        pss[k][:, 0:1024],
                        None,
                        1024,
                    )
            levels_deep(NPAIR - 1, ustrbs[NPAIR - 1], ustr8s[NPAIR - 1], emb10sh)

            # ---- levels 9..1 in two independent 4-pair chains ----
            # Levels 9..8 use the fused fp8 L+R (inputs emb10sh/esh9 are fp8);
            # levels 7..1 are bf16 (fp8 noise near the root does not average).
            eshs = [{}, {}]
            for i in range(9, 0, -1):
                m4 = 4 * (2**i)
                b8 = 8 * (2**i - 1)
                fused = i >= 8
                for X in range(2):
                    prev = emb10sh if i == 9 else eshs[X][i + 1]
                    pb = 4096 * X if i == 9 else 0
                    cur = wpool.tile(
                        [128, m4], fp8 if i == 9 else bf16, tag=f"esh{X}_{i}"
                    )
                    eshs[X][i] = cur
                    for s0 in range(0, m4, 1024):
                        w = min(1024, m4 - s0)
                        ps = pspool.tile(
                            [128, 1024], fp32, tag="ps", name=f"pst{X}_{i}_{s0}"
                        )
                        if fused:
                            for h0 in range(0, w, 512):
                                n = min(512, w - h0)
                                j0 = s0 + h0
                                mv = prev[
                                    :, pb + 2 * j0 : pb + 2 * j0 + 2 * n
                                ].rearrange("p (n two) -> p two n", two=2)
                                nc.tensor.matmul(
                                    ps[:, h0 : h0 + n],
                                    whlr_v,
                                    mv,
                                    start=True,
                                    stop=False,
                                    perf_mode=DR,
                                )
                            for h0 in range(0, w, 512):
                                n = min(512, w - h0)
                                j0 = s0 + h0
                                nc.tensor.matmul(
                                    ps[:, h0 : h0 + n],
                                    whub_sb[:],
                                    utop[:, b8 + m4 * X + j0 : b8 + m4 * X + j0 + n],
                                    start=False,
                                    stop=True,
                                )
                        else:
                            for w_sb, kind in (
                                (whlb_sb, "L"),
                                (whrb_sb, "R"),
                                (whub_sb, "U"),
                            ):
                                for h0 in range(0, w, 512):
                                    n = min(512, w - h0)
                                    j0 = s0 + h0
                                    if kind == "L":
                                        mv = prev[
                                            :, pb + 2 * j0 : pb + 2 * j0 + 2 * n : 2
                                        ]
                                    elif kind == "R":
                                        mv = prev[
                                            :,
                                            pb + 2 * j0 + 1 : pb + 2 * j0 + 2 * n : 2,
                                        ]
                                    else:
                                        mv = utop[
                                            :, b8 + m4 * X + j0 : b8 + m4 * X + j0 + n
                                        ]
                                    nc.tensor.matmul(
                                        ps[:, h0 : h0 + n],
                                        w_sb[:],
                                        mv,
                                        start=(kind == "L"),
                                        stop=(kind == "U"),
                                    )
                        act_relu(cur[:, s0 : s0 + w], ps[:, 0:w], bh_sb[:], w)

            # ---- level 0: roots, one per chain ----
            roots = wpool.tile([128, NPAIR], fp32, tag="roots")
            for X in range(2):
                ps = pspool.tile([128, 1024], fp32, tag="ps", name=f"psroot{X}")
                o = ps[:, 0:4]
                e1 = eshs[X][1]
                nc.tensor.matmul(o, whlb_sb[:], e1[:, 0:8:2], start=True, stop=False)
                nc.tensor.matmul(o, whrb_sb[:], e1[:, 1:8:2], start=False, stop=False)
                nc.tensor.matmul(
                    o, whub_sb[:], utop[:, 4 * X : 4 * X + 4], start=False, stop=True
                )
                nc.scalar.activation(
                    roots[:, 4 * X : 4 * X + 4], o, RELU, bias=bh_sb[:]
                )
            nc.sync.dma_start(out_d.ap(), roots[:])

    _dedup_ldweights(nc)
    _split_sync_waits(nc, mybir)
    return nc


_NC_CACHE = None
LAST_RESULTS = None


def kernel(contents, children, Wu, bu, Wh, bh):
    global _NC_CACHE, LAST_RESULTS
    contents = np.asarray(contents, dtype=np.float32)
    children = np.asarray(children)
    Wu = np.asarray(Wu, dtype=np.float32)
    bu = np.asarray(bu, dtype=np.float32)
    Wh = np.asarray(Wh, dtype=np.float32)
    bh = np.asarray(bh, dtype=np.float32)

    regular = (
        contents.shape == (B, N_NODES, F)
        and children.shape == (N_INNER, 2)
        and np.array_equal(
            np.asarray(children, dtype=np.int64).ravel(), np.arange(N_INNER * 2)
        )
    )
    if not regular:
        # Safety net for non-arange children: exact numpy fallback.
        return _np_reference(contents, children, Wu, bu, Wh, bh)

    from concourse.bass_utils import run_bass_kernel_spmd

    if _NC_CACHE is None:
        _NC_CACHE = _build_nc()
    nc = _NC_CACHE

    wts = _prep_weights(Wu, bu, Wh, bh)
    in_maps = []
    for k in range(N_CORES):
        m = _prep_core_inputs(contents[JPC * k : JPC * (k + 1)])
        m.update(wts)
        in_maps.append(m)

    res = run_bass_kernel_spmd(
        nc,
        in_maps,
        core_ids=list(range(N_CORES)),
        trace=bool(os.environ.get("BASS_TRACE")),
    )
    LAST_RESULTS = res

    out = np.empty((B, H), dtype=np.float32)
    for k in range(N_CORES):
        r = res.results[k]["out"].reshape(2, 64, NPAIR)  # [half, h, pair]
        out[JPC * k : JPC * (k + 1)] = np.transpose(r, (2, 0, 1)).reshape(JPC, H)
    return out


# revision 42
# speedup vs baseline: 1.0341x; 1.0034x over previous
"""Trainium2 Bass kernel for GRNNTransformSimple (bottom-up binary-tree GRNN).

Computation (per jet): heap-layout complete binary tree, DEPTH=14.
  u_k   = relu(contents_k @ Wu + bu)                         (all nodes)
  emb_k = u_k                                                (leaves)
  emb_k = relu(hL @ Wh[:64] + hR @ Wh[64:128] + u_k @ Wh[128:] + bh)  (inner)
Output: root emb, [B, 64].

Mapping (8 NeuronCores, data-parallel over B=128 jets, 16 jets/core):
 - 2 jets packed per 128 SBUF partitions (jet A on partitions 0-63, jet B on
   64-127) with block-diagonal weights -> all engines run 128 partitions wide.
 - The PE streams exactly 1 output column per cycle in every mode
   (measured: bf16 = fp8 = fp8-DoubleRow = 216ns per 512-col matmul), so
   the only matmul-count lever is fusing the L and R contributions into a
   single pass: fp8e4m3 DoubleRow with stationary planes (WhL_bd, WhR_bd)
   and moving planes (hL, hR) = the even/odd column interleave of the
   child level. Applied at the deep levels (12..10, 87% of fc_h work).
   Deep-level fp8 quantization noise averages out on the way up the tree
   (measured: rel_rms 4.1e-3 vs 4.1e-3 pure-bf16 on the reference inputs);
   fp8 on the top levels does NOT average and is left in bf16.
 - fc_u biases folded into the matmul via a constant-one input row (K=18),
   4-way strip-tiled (tile_position) since K=18 << 128. Each 2048-col
   stream window is issued as a WAVE of 4 consecutive strip matmuls at
   different tile positions: the PE's 16x 32x32 sub-arrays execute them
   CONCURRENTLY (measured 53ns/matmul vs 216 serial, ~4x), so fc_u drops
   from 51us to ~13us of PE time. The deep fc_u stream (c4 + its weights)
   is fp8 (halves the startup DMA and SBUF reads; same PE speed).
 - fc_h U-term stays bf16 (same pass count as fp8, cheap fast-weight-load).
 - Levels 9..0 are batched across pairs, split into two independent 4-pair
   chains so one chain's matmuls fill the other's activation stalls;
   levels 9..8 use the fused fp8 L+R, 7..0 bf16 (top-level u stays bf16).
 - The "irregular" child gather is regular for arange children: children of
   level-i node j are nodes 2j, 2j+1 of level i+1, i.e. stride-2 column
   slices (the fp8 DoubleRow plane view) of the level-(i+1) embedding.
 - Pair 0's c4 is DMA'd in 128KB column chunks so the first wave starts
   ~1us in; later pairs use one DMA each (Sync-engine issue cost, ~600ns
   per DMA, dominates over transfer bandwidth). The u phase for levels
   0..9 (utop) is emitted last to fill tail stalls.
 - relu activations are split between the Scalar and Vector engines by a
   greedy balance on estimated cost; with the PE tricks above the act
   engines are co-critical with the PE (~110us busy each).
"""

import os
import sys

sys.path.insert(0, "/opt/trn_rl_repo")

import ml_dtypes
import numpy as np

DEPTH = 14
B = 128
F = 8
H = 64
N_NODES = 2**DEPTH - 1  # 16383
N_INNER = 2 ** (DEPTH - 1) - 1  # 8191
N_CORES = 8
JPC = 16  # jets per core
NPAIR = 8  # jet pairs per core

BF16 = ml_dtypes.bfloat16
FP8 = ml_dtypes.float8_e4m3fn

# u_stream layout per pair (columns): levels 10,11,12 inner nodes in heap
# order, then all leaves in heap order.
UB10, UB11, UB12 = 0, 1024, 3072  # level bases inside u_stream
ULEAF = 7168
USTREAM = 15360  # 1024 + 2048 + 4096 + 8192
NGRP = 15  # 15 groups x 1024 cols
# u_top: levels 0..9, column order [level][pair][node]
UTOP_COLS = 8184  # 8 * 1023
UTOP_PAD = 8192


def _np_reference(contents, children, Wu, bu, Wh, bh):
    emb = None
    for i in range(DEPTH - 1, -1, -1):
        off, n = 2**i - 1, 2**i
        u = np.maximum(contents[:, off : off + n] @ Wu + bu, 0)
        if emb is None:
            emb = u
        else:
            ch = children[off : off + n] - 2 * off
            hL = emb[:, ch[:, 0]]
            hR = emb[:, ch[:, 1]]
            emb = np.maximum(
                hL @ Wh[:H] + hR @ Wh[H : 2 * H] + u @ Wh[2 * H :] + bh, 0
            )
    return emb.reshape(emb.shape[0], -1).astype(np.float32)


def _strip_pack(S):
    """Pack a [18, N] stream into the [128, ceil(N/4096)*1024] strip layout:
    wave (j, h) covers the contiguous 2048-col stream window starting at
    4096j + 2048h; its 4 strips (rows 32t..32t+17, tile cols
    [1024j + 512h : +512]) hold the window's four 512-col quarters. The 4
    strip matmuls of a wave then run concurrently in the PE sub-arrays and
    fill one [128, 2048] PSUM tile that drains with a single activation."""
    n = S.shape[1]
    out = np.zeros((128, ((n + 4095) // 4096) * 1024), dtype=S.dtype)
    for w0 in range(0, n, 2048):  # wave window
        j, h = w0 // 4096, (w0 % 4096) // 2048
        for t in range(4):
            s0 = w0 + 512 * t
            if s0 >= n:
                break
            s1 = min(s0 + 512, n)
            c0 = 1024 * j + 512 * h
            out[32 * t : 32 * t + 18, c0 : c0 + (s1 - s0)] = S[:, s0:s1]
    return out


def _prep_core_inputs(contents):
    """contents: [16, 16383, 8] f32 for one core.
    Returns dict of per-core device input arrays."""
    c4 = np.zeros((NPAIR, 128, 4096), dtype=FP8)
    big_T = np.ascontiguousarray(
        np.transpose(contents[:, 1023:16383, :], (0, 2, 1))
    )  # [16, 8, 15360]
    for p in range(NPAIR):
        S = np.empty((18, USTREAM), dtype=np.float32)
        S[0:8] = big_T[2 * p]
        S[8] = 1.0
        S[9:17] = big_T[2 * p + 1]
        S[17] = 1.0
        c4[p] = _strip_pack(S.astype(FP8))

    # u_top stream: [level][pair][node]
    tops = np.empty((18, UTOP_COLS), dtype=np.float32)
    colptr = 0
    cT = np.transpose(contents, (0, 2, 1))  # [16, 8, 16383]
    for i in range(10):
        off, n = 2**i - 1, 2**i
        for p in range(NPAIR):
            tops[0:8, colptr : colptr + n] = cT[2 * p][:, off : off + n]
            tops[8, colptr : colptr + n] = 1.0
            tops[9:17, colptr : colptr + n] = cT[2 * p + 1][:, off : off + n]
            tops[17, colptr : colptr + n] = 1.0
            colptr += n
    assert colptr == UTOP_COLS
    tb = np.zeros((18, UTOP_PAD), dtype=BF16)
    tb[:, :UTOP_COLS] = tops.astype(BF16)
    ctop = _strip_pack(tb)
    return {"c4": c4, "ctop": ctop}


def _prep_weights(Wu, bu, Wh, bh):
    wu2 = np.zeros((18, 128), dtype=np.float32)
    wu2[0:8, 0:64] = Wu
    wu2[8, 0:64] = bu
    wu2[9:17, 64:128] = Wu
    wu2[17, 64:128] = bu
    # fc_u stationary, 4-way strip layout: fp8 for the deep stream,
    # bf16 for the top-levels (ctop) stream
    wub = np.zeros((128, 128), dtype=BF16)
    wu8 = np.zeros((128, 128), dtype=FP8)
    for t in range(4):
        wub[32 * t : 32 * t + 18, :] = wu2.astype(BF16)
        wu8[32 * t : 32 * t + 18, :] = wu2.astype(FP8)

    def blockdiag(Wx):
        out = np.zeros((128, 128), dtype=np.float32)
        out[0:64, 0:64] = Wx
        out[64:128, 64:128] = Wx
        return out

    bdl = blockdiag(Wh[0:H])
    bdr = blockdiag(Wh[H : 2 * H])
    bdu = blockdiag(Wh[2 * H : 3 * H])
    whlr8 = np.concatenate([bdl, bdr], axis=1).astype(FP8)
    bh2 = np.concatenate([bh, bh]).astype(np.float32).reshape(128, 1)
    return {
        "wub": wub,
        "wu8": wu8,
        "whlr8": whlr8,
        "whlb": bdl.astype(BF16),
        "whrb": bdr.astype(BF16),
        "whub": bdu.astype(BF16),
        "bh2": bh2,
    }


def _dedup_ldweights(nc):
    """Delete an LDWEIGHTS whose signature matches the last weight load FOR
    ITS TILE POSITION (only MATMULs in between): the 128x128 PE array is 16
    independent 32x32 sub-arrays, so each tile position keeps its stationary
    operand resident until an overlapping load clobbers it. Sync info of
    deleted loads is merged into the following PE instruction."""
    n_del = 0

    def row_range(inst):
        pos = inst.tile_position
        size = inst.tile_size
        r0 = pos[0] if pos else 0
        nr = size[0] if size else 128
        return (r0, r0 + nr)

    for f in nc.m.functions:
        for bb in f.blocks:
            last_by_pos = {}  # (r0, r1) -> sig
            pending_sync = None
            out = []
            for inst in bb.instructions:
                tn = type(inst).__name__
                if str(getattr(inst, "engine", "")) == "EngineType.PE":
                    if tn == "InstLdweights":
                        a = inst.ins[0]
                        rr = row_range(inst)
                        sig = (
                            getattr(a, "memref", None),
                            getattr(a, "offset", None),
                            str(getattr(a, "ap", None)),
                            str(inst.tile_position),
                            str(inst.tile_size),
                            str(inst.perf_mode),
                            str(inst.is_transpose),
                        )
                        if last_by_pos.get(rr) == sig:
                            n_del += 1
                            si = inst.sync_info
                            if si is not None and (si.on_wait or si.on_update):
                                if pending_sync is None:
                                    pending_sync = ([], [])
                                pending_sync[0].extend(si.on_wait)
                                pending_sync[1].extend(si.on_update)
                            continue  # drop this instruction
                        # clobber any overlapping row range, then record
                        for orr in [
                            k
                            for k in last_by_pos
                            if k[0] < rr[1] and rr[0] < k[1] and k != rr
                        ]:
                            del last_by_pos[orr]
                        last_by_pos[rr] = sig
                    elif tn != "InstMatmult":
                        last_by_pos.clear()  # anything else on PE invalidates
                    if pending_sync is not None:
                        si = inst.sync_info
                        if si is None:
                            import concourse.mybir as mybir

                            inst.sync_info = mybir.SyncInfo(
                                on_wait=list(pending_sync[0]),
                                on_update=list(pending_sync[1]),
                            )
                        else:
                            si.on_wait[:0] = pending_sync[0]
                            si.on_update.extend(pending_sync[1])
                        pending_sync = None
                out.append(inst)
            assert pending_sync is None, "dangling sync from deleted trailing LDW"
            bb.instructions.clear()
            for i in out:
                bb.add_instruction(i)
    return n_del


def _split_sync_waits(nc, mybir, max_waits=1):
    """This container's walrus only accepts 1 sync-wait per instruction;
    move excess waits onto preceding same-engine NoOps."""
    for f in nc.m.functions:
        for bb in f.blocks:
            out = []
            for inst in bb.instructions:
                si = inst.sync_info
                if si is not None and len(si.on_wait) > max_waits:
                    waits = list(si.on_wait)
                    extra, keep = waits[:-max_waits], waits[-max_waits:]
                    for i in range(0, len(extra), max_waits):
                        nop = mybir.InstNoOp(
                            name=nc.get_next_instruction_name(),
                            engine=inst.engine,
                            sync_info=mybir.SyncInfo(
                                on_wait=extra[i : i + max_waits], on_update=[]
                            ),
                        )
                        out.append(nop)
                    si.on_wait = keep
                out.append(inst)
            bb.instructions.clear()
            for i in out:
                bb.add_instruction(i)


def _build_nc():
    import concourse.bass as bass
    import concourse.mybir as mybir
    from concourse.tile import TileContext

    fp32 = mybir.dt.float32
    bf16 = mybir.dt.bfloat16
    fp8 = mybir.dt.float8e4
    RELU = mybir.ActivationFunctionType.Relu
    ADD = mybir.AluOpType.add
    MAX = mybir.AluOpType.max
    DR = mybir.MatmulPerfMode.DoubleRow

    nc = bass.Bass(trn_type="TRN2", num_devices=N_CORES)
    c4_d = nc.dram_tensor("c4", [NPAIR, 128, 4096], fp8, kind="ExternalInput")
    ctop_d = nc.dram_tensor("ctop", [128, 2048], bf16, kind="ExternalInput")
    wub_d = nc.dram_tensor("wub", [128, 128], bf16, kind="ExternalInput")
    wu8_d = nc.dram_tensor("wu8", [128, 128], fp8, kind="ExternalInput")
    whlr8_d = nc.dram_tensor("whlr8", [128, 256], fp8, kind="ExternalInput")
    whlb_d = nc.dram_tensor("whlb", [128, 128], bf16, kind="ExternalInput")
    whrb_d = nc.dram_tensor("whrb", [128, 128], bf16, kind="ExternalInput")
    whub_d = nc.dram_tensor("whub", [128, 128], bf16, kind="ExternalInput")
    bh2_d = nc.dram_tensor("bh2", [128, 1], fp32, kind="ExternalInput")
    out_d = nc.dram_tensor("out", [128, NPAIR], fp32, kind="ExternalOutput")

    # greedy act-engine balance: [scalar(ACT), vector(DVE)] cumulative ns
    eng_load = [0.0, 0.0]

    with TileContext(nc) as tc:
        with (
            tc.tile_pool(name="wpool", bufs=1) as wpool,
            tc.tile_pool(name="c4pool", bufs=3) as c4pool,
            tc.tile_pool(name="uspool", bufs=4) as uspool,
            tc.tile_pool(name="e12pool", bufs=2) as e12pool,
            tc.tile_pool(name="e11pool", bufs=2) as e11pool,
            tc.tile_pool(name="shpool", bufs=1) as shpool,
            tc.tile_pool(name="pspool", bufs=4, space="PSUM") as pspool,
        ):
            whlr_sb = wpool.tile([128, 256], fp8, tag="whlr")
            bh_sb = wpool.tile([128, 1], fp32, tag="bh")
            wub_sb = wpool.tile([128, 128], bf16, tag="wub")
            wu8_sb = wpool.tile([128, 128], fp8, tag="wu8")
            whlb_sb = wpool.tile([128, 128], bf16, tag="whlb")
            whrb_sb = wpool.tile([128, 128], bf16, tag="whrb")
            whub_sb = wpool.tile([128, 128], bf16, tag="whub")
            ctop_sb = wpool.tile([128, 2048], bf16, tag="ctop")
            utop = wpool.tile([128, UTOP_PAD], bf16, tag="utop")
            # only the fc_u weights gate the first wave; everything else is
            # issued after pair 0's chunks (Sync DMA issue is ~600ns each)
            nc.sync.dma_start(wu8_sb[:], wu8_d.ap())

            whlr_v = whlr_sb[:, 0:256].rearrange("p (two m) -> p two m", two=2)

            def act_relu(dst_ap, src_ap, bias, ncols):
                """relu(src + bias) -> dst on the act engine with the least
                estimated accumulated load."""
                cost = (230.0 + 0.833 * ncols, 147.0 + 1.042 * ncols)
                e = 0 if eng_load[0] + cost[0] <= eng_load[1] + cost[1] else 1
                eng_load[e] += cost[e]
                if e == 0:
                    if bias is None:
                        nc.scalar.activation(dst_ap, src_ap, RELU)
                    else:
                        nc.scalar.activation(dst_ap, src_ap, RELU, bias=bias)
                else:
                    if bias is None:
                        nc.vector.tensor_scalar(dst_ap, src_ap, 0.0, None, MAX)
                    else:
                        nc.vector.tensor_scalar(dst_ap, src_ap, bias, 0.0, ADD, MAX)

            def u_phase(p, chunks, ustrb, ustr8):
                """fc_u for the deep stream of one pair (bf16, strip-tiled).
                Each wave = 4 matmuls at different tile_positions, run
                CONCURRENTLY by the PE's 32-row sub-arrays (~4x throughput),
                filling the four 512-col quarters of ONE [128, 2048] PSUM
                tile = one contiguous stream window = one activation.
                Inner-node u (cols < 7168) lands in bf16 for the U-term;
                leaf u lands in fp8 for the level-12 fused L+R."""
                for w in range(8):
                    s0 = 2048 * w
                    nstrip = min(4, (USTREAM - s0 + 511) // 512)
                    ch, cb = chunks[w // 2]
                    hc = cb + 512 * (w % 2)
                    # wave of up-to-4 concurrent strip matmuls filling two
                    # [128, 1024] psum tiles (strips 0-1 / 2-3)
                    pss = [
                        pspool.tile(
                            [128, 1024], fp32, tag="ps", name=f"psu{p}_{w}_{k}"
                        )
                        for k in range((nstrip + 1) // 2)
                    ]
                    for t in range(nstrip):
                        nc.tensor.matmul(
                            pss[t // 2][:, 512 * (t % 2) : 512 * (t % 2 + 1)],
                            wu8_sb[32 * t : 32 * t + 18, :],
                            ch[32 * t : 32 * t + 18, hc : hc + 512],
                            start=True,
                            stop=True,
                            tile_position=(32 * t, 0),
                        )
                    for k in range((nstrip + 1) // 2):
                        a0 = s0 + 1024 * k
                        a1 = min(a0 + 1024, s0 + 512 * nstrip)
                        if a0 < ULEAF:
                            dstt, base = ustrb, 0
                        else:
                            dstt, base = ustr8, ULEAF
                        act_relu(
                            dstt[:, a0 - base : a1 - base],
                            pss[k][:, 0 : a1 - a0],
                            None,
                            a1 - a0,
                        )

            def levels_deep(p, ustrb, ustr8, emb10sh):
                """fc_h levels 12..10 for one pair: fused L+R via fp8
                DoubleRow, U-term in bf16, 2048-col supergroups -> one act
                per supergroup and 2 weight switches."""
                emb12 = e12pool.tile([128, 4096], fp8, tag="e12")
                emb11 = e11pool.tile([128, 2048], fp8, tag="e11")
                for i, ubase, prev, dst, dst_base in (
                    (12, UB12, ustr8, emb12, 0),
                    (11, UB11, emb12, emb11, 0),
                    (10, UB10, emb11, emb10sh, 1024 * p),
                ):
                    m = 2**i
                    groups = list(range(0, m, 1024))
                    for g0 in range(0, len(groups), 2):
                        grp = groups[g0 : g0 + 2]
                        pss = [
                            pspool.tile(
                                [128, 1024], fp32, tag="ps", name=f"psl{p}_{i}_{s0}"
                            )
                            for s0 in grp
                        ]
                        for ci, s0 in enumerate(grp):
                            w = min(1024, m - s0)
                            for h0 in range(0, w, 512):
                                j0 = s0 + h0
                                mv = prev[:, 2 * j0 : 2 * j0 + 1024].rearrange(
                                    "p (n two) -> p two n", two=2
                                )
                                nc.tensor.matmul(
                                    pss[ci][:, h0 : h0 + 512],
                                    whlr_v,
                                    mv,
                                    start=True,
                                    stop=False,
                                    perf_mode=DR,
                                )
                        for ci, s0 in enumerate(grp):
                            w = min(1024, m - s0)
                            for h0 in range(0, w, 512):
                                j0 = s0 + h0
                                nc.tensor.matmul(
                                    pss[ci][:, h0 : h0 + 512],
                                    whub_sb[:],
                                    ustrb[:, ubase + j0 : ubase + j0 + 512],
                                    start=False,
                                    stop=True,
                                )
                        for ci, s0 in enumerate(grp):
                            w = min(1024, m - s0)
                            act_relu(
                                dst[:, dst_base + s0 : dst_base + s0 + w],
                                pss[ci][:, 0:w],
                                bh_sb[:],
                                w,
                            )

            # ---- pairs, software-pipelined: u(p) emitted before levels(p-1)
            # so independent u work fills the level chains' act stalls ----
            emb10sh = shpool.tile([128, 8192], fp8, tag="e10")
            ustrbs = [None] * NPAIR
            ustr8s = [None] * NPAIR
            for p in range(NPAIR):
                chunks = []
                if p == 0:
                    # chunked first pair so the first wave starts ~1us in
                    for j in range(4):
                        ch = c4pool.tile(
                            [128, 1024], fp8, tag="c4", name=f"c4_{p}_{j}"
                        )
                        nc.sync.dma_start(
                            ch[:], c4_d.ap()[p][:, 1024 * j : 1024 * (j + 1)]
                        )
                        chunks.append((ch, 0))
                else:
                    # one DMA per pair: issue cost on the Sync engine is the
                    # startup bottleneck, not transfer bandwidth
                    whole = c4pool.tile([128, 4096], fp8, tag="c4w", name=f"c4w{p}")
                    nc.sync.dma_start(whole[:], c4_d.ap()[p])
                    chunks = [(whole, 1024 * j) for j in range(4)]
                if p == 1:
                    # weights for the deep/top phases + tail inputs: issued
                    # after pair 1's contents so they don't delay the u(1)
                    # waves that hide u(0)'s activation drain
                    nc.sync.dma_start(whlr_sb[:], whlr8_d.ap())
                    nc.sync.dma_start(whub_sb[:], whub_d.ap())
                    nc.sync.dma_start(bh_sb[:], bh2_d.ap())
                    nc.sync.dma_start(wub_sb[:], wub_d.ap())
                    nc.sync.dma_start(whlb_sb[:], whlb_d.ap())
                    nc.sync.dma_start(whrb_sb[:], whrb_d.ap())
                    nc.sync.dma_start(ctop_sb[:], ctop_d.ap())
                ustrbs[p] = uspool.tile([128, 7168], bf16, tag="usb", name=f"ustrb{p}")
                ustr8s[p] = uspool.tile([128, 8192], fp8, tag="us8", name=f"ustr8{p}")
                u_phase(p, chunks, ustrbs[p], ustr8s[p])
                if p > 0:
                    levels_deep(p - 1, ustrbs[p - 1], ustr8s[p - 1], emb10sh)

            # ---- u for levels 0..9 (strip waves): emitted before the last
            # deep pair so its activations drain during the deep phase and
            # the top chains can start the moment emb10sh completes ----
            for w in range(4):
                cc = 1024 * (w // 2) + 512 * (w % 2)
                pss = [
                    pspool.tile([128, 1024], fp32, tag="ps", name=f"psut{w}_{k}")
                    for k in range(2)
                ]
                for t in range(4):
                    nc.tensor.matmul(
                        pss[t // 2][:, 512 * (t % 2) : 512 * (t % 2 + 1)],
                        wub_sb[32 * t : 32 * t + 18, :],
                        ctop_sb[32 * t : 32 * t + 18, cc : cc + 512],
                        start=True,
                        stop=True,
                        tile_position=(32 * t, 0),
                    )
                for k in range(2):
                    act_relu(
                        utop[:, 2048 * w + 1024 * k : 2048 * w + 1024 * (k + 1)],
                        pss[k][:, 0:1024],
                        None,
                        1024,
                    )
            levels_deep(NPAIR - 1, ustrbs[NPAIR - 1], ustr8s[NPAIR - 1], emb10sh)

            # ---- levels 9..1 in two independent 4-pair chains ----
            # Levels 9..8 use the fused fp8 L+R (inputs emb10sh/esh9 are fp8);
            # levels 7..1 are bf16 (fp8 noise near the root does not average).
            eshs = [{}, {}]
            for i in range(9, 0, -1):
                m4 = 4 * (2**i)
                b8 = 8 * (2**i - 1)
                fused = i >= 8
                for X in range(2):
                    prev = emb10sh if i == 9 else eshs[X][i + 1]
                    pb = 4096 * X if i == 9 else 0
                    cur = wpool.tile(
                        [128, m4], fp8 if i == 9 else bf16, tag=f"esh{X}_{i}"
                    )
                    eshs[X][i] = cur
                    for s0 in range(0, m4, 1024):
                        w = min(1024, m4 - s0)
                        ps = pspool.tile(
                            [128, 1024], fp32, tag="ps", name=f"pst{X}_{i}_{s0}"
                        )
                        if fused:
                            for h0 in range(0, w, 512):
                                n = min(512, w - h0)
                                j0 = s0 + h0
                                mv = prev[
                                    :, pb + 2 * j0 : pb + 2 * j0 + 2 * n
                                ].rearrange("p (n two) -> p two n", two=2)
                                nc.tensor.matmul(
                                    ps[:, h0 : h0 + n],
                                    whlr_v,
                                    mv,
                                    start=True,
                                    stop=False,
                                    perf_mode=DR,
                                )
                            for h0 in range(0, w, 512):
                                n = min(512, w - h0)
                                j0 = s0 + h0
                                nc.tensor.matmul(
                                    ps[:, h0 : h0 + n],
                                    whub_sb[:],
                                    utop[:, b8 + m4 * X + j0 : b8 + m4 * X + j0 + n],
                                    start=False,
                                    stop=True,
                                )
                        else:
                            for w_sb, kind in (
                                (whlb_sb, "L"),
                                (whrb_sb, "R"),
                                (whub_sb, "U"),
                            ):
                                for h0 in range(0, w, 512):
                                    n = min(512, w - h0)
                                    j0 = s0 + h0
                                    if kind == "L":
                                        mv = prev[
                                            :, pb + 2 * j0 : pb + 2 * j0 + 2 * n : 2
                                        ]
                                    elif kind == "R":
                                        mv = prev[
                                            :,
                                            pb + 2 * j0 + 1 : pb + 2 * j0 + 2 * n : 2,
                                        ]
                                    else:
                                        mv = utop[
                                            :, b8 + m4 * X + j0 : b8 + m4 * X + j0 + n
                                        ]
                                    nc.tensor.matmul(
                                        ps[:, h0 : h0 + n],
                                        w_sb[:],
                                        mv,
                                        start=(kind == "L"),
                                        stop=(kind == "U"),
                                    )
                        act_relu(cur[:, s0 : s0 + w], ps[:, 0:w], bh_sb[:], w)

            # ---- level 0: roots, one per chain ----
            roots = wpool.tile([128, NPAIR], fp32, tag="roots")
            for X in range(2):
                ps = pspool.tile([128, 1024], fp32, tag="ps", name=f"psroot{X}")
                o = ps[:, 0:4]
                e1 = eshs[X][1]
                nc.tensor.matmul(o, whlb_sb[:], e1[:, 0:8:2], start=True, stop=False)
                nc.tensor.matmul(o, whrb_sb[:], e1[:, 1:8:2], start=False, stop=False)
                nc.tensor.matmul(
                    o, whub_sb[:], utop[:, 4 * X : 4 * X + 4], start=False, stop=True
                )
                nc.scalar.activation(
                    roots[:, 4 * X : 4 * X + 4], o, RELU, bias=bh_sb[:]
                )
            nc.sync.dma_start(out_d.ap(), roots[:])

    _dedup_ldweights(nc)
    _split_sync_waits(nc, mybir)
    return nc


_NC_CACHE = None
LAST_RESULTS = None


def kernel(contents, children, Wu, bu, Wh, bh):
    global _NC_CACHE, LAST_RESULTS
    contents = np.asarray(contents, dtype=np.float32)
    children = np.asarray(children)
    Wu = np.asarray(Wu, dtype=np.float32)
    bu = np.asarray(bu, dtype=np.float32)
    Wh = np.asarray(Wh, dtype=np.float32)
    bh = np.asarray(bh, dtype=np.float32)

    regular = (
        contents.shape == (B, N_NODES, F)
        and children.shape == (N_INNER, 2)
        and np.array_equal(
            np.asarray(children, dtype=np.int64).ravel(), np.arange(N_INNER * 2)
        )
    )
    if not regular:
        # Safety net for non-arange children: exact numpy fallback.
        return _np_reference(contents, children, Wu, bu, Wh, bh)

    from concourse.bass_utils import run_bass_kernel_spmd

    if _NC_CACHE is None:
        _NC_CACHE = _build_nc()
    nc = _NC_CACHE

    wts = _prep_weights(Wu, bu, Wh, bh)
    in_maps = []
    for k in range(N_CORES):
        m = _prep_core_inputs(contents[JPC * k : JPC * (k + 1)])
        m.update(wts)
        in_maps.append(m)

    res = run_bass_kernel_spmd(
        nc,
        in_maps,
        core_ids=list(range(N_CORES)),
        trace=bool(os.environ.get("BASS_TRACE")),
    )
    LAST_RESULTS = res

    out = np.empty((B, H), dtype=np.float32)
    for k in range(N_CORES):
        r = res.results[k]["out"].reshape(2, 64, NPAIR)  # [half, h, pair]
        out[JPC * k : JPC * (k + 1)] = np.transpose(r, (2, 0, 1)).reshape(JPC, H)
    return out
